# revision 46
# baseline (speedup 1.0000x reference)
"""BERT-CRF NER Viterbi decode kernel for Trainium2 (8 NeuronCores).

Strategy (data-parallel over batch, 8 rows/core), raw Bass:
  - host: shard hidden_states [64,512,768] -> 8 x [8,512,768], pre-transpose
    to [8,768,512] and cast to bf16 (halves the dominant HBM read; validated
    ~3e-3 path mismatch, far under the 2e-2 gate). W compact+bf16.
  - device (per core):
      feats = W.T @ hsT per batch row -> PSUM [8,512] (6 K-chunks, bf16 PE,
        inputs streamed in kc-pair chunks over 3 DMA queues: SP/ACT/Pool)
      ACT copies PSUM->SBUF stage; DMA spreads feats to a chunked layout
        feat_sp[p = c*8+b, (to,t_local)]  (C=16 time-chunks of S=32 steps)
      Tree-compose (max,+) pair products per chunk in fp16 (TensorTensor
        runs 2x on packed fp16; products kept in both orientations so every
        operand is unit-stride): level 0 decomposes A = trep + feat into
        G3[j,to,k] = feat[2j+1,to]+feat[2j,k]+trep[to,k] then composes with
        the constant trep^T; levels 1..4 pair up products to the chunk
        product E.  Chunk-0 pair 0 is overwritten with the tropical
        identity (the uniform recurrence starts at t=2 with carry delta_1).
      Carry chain: gather E to [b, c] layout via a DRAM bounce, 15 serial
        matrix-vector steps D_{c+1} = E_c (x) D_c (f32), scatter back.
      Phase 3 re-scan per chunk: 8 serial quad steps + two bulk fills
        (pairs, then evens via f32 A matrices) -> delta_t for all t (f32).
  - host: final-step argmax from f511 dump, psi + backtrace from delta
    (identical argmax semantics to the reference; restricted to from-labels
    0..6 which provably always win).
"""

import numpy as np
from contextlib import ExitStack

import concourse.bass as bass
from concourse import mybir
from concourse.bass_utils import run_bass_kernel_spmd

B, T, H, L = 64, 512, 768, 9
NC = 8              # cores
BL = B // NC        # batch rows per core = 8
KC = H // 128       # 6 contraction chunks
C = 16              # time chunks per sequence
S = T // C          # 32 steps per chunk
NP = S // 2         # 16 pairs per chunk
START = 7
NEG = -10000.0

F32 = mybir.dt.float32
F16 = mybir.dt.float16
BF16 = mybir.dt.bfloat16
ADD = mybir.AluOpType.add
MAX = mybir.AluOpType.max
AXX = mybir.AxisListType.X

LC = 8          # compact 'to' labels: (0..6, 8); START row dropped
FC = 7          # compact 'from' labels: 0..6
LAB = [0, 1, 2, 3, 4, 5, 6, 8]


def build_program():
    nc = bass.Bass("TRN2", target_bir_lowering=False,
                   detect_race_conditions=False)

    hsT_d = nc.dram_tensor("hsT", [BL, H, T], BF16, kind="ExternalInput")
    wk_d = nc.dram_tensor("wk", [128, KC * LC], BF16, kind="ExternalInput")
    trep7_d = nc.dram_tensor("trep7", [128, FC * FC], F32,
                             kind="ExternalInput")
    trep7h_d = nc.dram_tensor("trep7h", [128, FC * FC], F16,
                              kind="ExternalInput")
    trep7t_d = nc.dram_tensor("trep7t", [128, FC * FC], F16,
                              kind="ExternalInput")
    d7c_d = nc.dram_tensor("d7c", [BL, FC], F32, kind="ExternalInput")
    identB_d = nc.dram_tensor("identB", [BL, FC * FC], F16,
                              kind="ExternalInput")
    # bounce buffers for cross-partition regroups
    eg_d = nc.dram_tensor("egb", [128, FC * FC], F16, kind="Internal")
    dg_d = nc.dram_tensor("dgb", [128, FC], F32, kind="Internal")
    st_d = nc.dram_tensor("stb", [128, LC * S], F32, kind="Internal")
    ddel_d = nc.dram_tensor("ddel", [128, S * FC], F32,
                            kind="ExternalOutput")
    f511_d = nc.dram_tensor("f511", [LC, BL], F32, kind="ExternalOutput")

    with ExitStack() as ctx:
        def sb(name, shape, dt=F32):
            return ctx.enter_context(nc.sbuf_tensor(name, shape, dt))
        wk = sb("wk_sb", [128, KC * LC], BF16)
        trep7 = sb("trep7_sb", [128, FC * FC])
        trep7h = sb("trep7h_sb", [128, FC * FC], F16)
        trep7t = sb("trep7t_sb", [128, FC * FC], F16)
        d7c = sb("d7c_sb", [BL, FC])
        identB = sb("identB_sb", [BL, FC * FC], F16)
        ht = [sb(f"ht{i}", [128, KC * T], BF16) for i in range(BL)]
        stage = sb("stage", [LC, BL * T])
        feat_sp = sb("feat_sp", [128, LC * S])
        feat_tp = sb("feat_tp", [128, S * LC], F16)   # [t, to] fp16
        A = sb("A_sb", [128, S * FC * FC])            # f32, fills only
        F2 = sb("F2", [128, NP * FC * FC], F16)
        G3 = sb("G3", [128, NP * FC * FC], F16)
        sw16 = sb("sw16", [128, NP * FC * FC * FC], F16)
        scw = sb("scw", [128, NP * FC * FC])          # f32 fill scratch
        Bp = sb("Bp", [128, NP * FC * FC], F16)       # pair products
        BpT = sb("BpT", [128, NP * FC * FC], F16)
        T2 = sb("T2", [128, 8 * FC * FC], F16)
        T2T = sb("T2T", [128, 8 * FC * FC], F16)
        T3 = sb("T3", [128, 4 * FC * FC], F16)
        T3T = sb("T3T", [128, 4 * FC * FC], F16)
        T4 = sb("T4", [128, 2 * FC * FC], F16)
        T4T = sb("T4T", [128, 2 * FC * FC], F16)
        Ee = sb("Ee", [128, FC * FC], F16)            # chunk product
        Eg = sb("Eg", [BL, C * FC * FC], F16)         # gathered [b, c]
        Dg = sb("Dg", [BL, C * FC])                   # carries [b, c] f32
        sc2 = sb("sc2", [BL, FC * FC])
        delta = sb("delta", [128, (S + 1) * FC])      # slot i = local i-1
        psum = [ctx.enter_context(nc.psum_tensor(f"psum{b}", [LC, T], F32))
                for b in range(BL)]

        in_sem = ctx.enter_context(nc.semaphore("in_sem"))
        wk_sem = ctx.enter_context(nc.semaphore("wk_sem"))
        hs_sems = [ctx.enter_context(nc.semaphore(f"hs_sem{i}"))
                   for i in range(BL)]
        pe_sem = ctx.enter_context(nc.semaphore("pe_sem"))
        cp_sem = ctx.enter_context(nc.semaphore("cp_sem"))
        sp_sem = ctx.enter_context(nc.semaphore("sp_sem"))
        ev_sem = ctx.enter_context(nc.semaphore("ev_sem"))
        g_sem = ctx.enter_context(nc.semaphore("g_sem"))
        p2_sem = ctx.enter_context(nc.semaphore("p2_sem"))
        sct_sem = ctx.enter_context(nc.semaphore("sct_sem"))
        dv_sem = ctx.enter_context(nc.semaphore("dv_sem"))
        out_sem = ctx.enter_context(nc.semaphore("out_sem"))
        block = ctx.enter_context(nc.Block())

        def ht_load(eng, b, kcp):
            """Load kc-pair chunk kcp of batch row b (PE streams behind)."""
            src = (hsT_d[b, :, :].rearrange("(kc p) t -> p kc t", p=128)
                   [:, 2 * kcp:2 * kcp + 2, :])
            dst = (ht[b][:, :].rearrange("p (kc t) -> p kc t", kc=KC)
                   [:, 2 * kcp:2 * kcp + 2, :])
            eng.dma_start(dst, src).then_inc(hs_sems[b], 16)

        @block.sync
        def _(sync):
            # wk first so the PE can start ASAP; hs chunks striped across
            # the 3 DMA queues (chunk q of each row on queue q) so row b
            # lands ~(b+1) transfer-times in, pipelining the PE perfectly
            sync.dma_start(wk[:, :], wk_d[:, :]).then_inc(wk_sem, 16)
            for b in range(BL):
                ht_load(sync, b, 0)
            sync.dma_start(trep7[:, :], trep7_d[:, :]).then_inc(in_sem, 16)
            sync.dma_start(trep7h[:, :], trep7h_d[:, :]).then_inc(in_sem, 16)
            sync.dma_start(trep7t[:, :], trep7t_d[:, :]).then_inc(in_sem, 16)
            sync.dma_start(d7c[:, :], d7c_d[:, :]).then_inc(in_sem, 16)
            sync.dma_start(identB[:, :], identB_d[:, :]).then_inc(in_sem, 16)
            # gather chunk products E[(c,b)] -> Eg[b, (c,...)] via DRAM
            sync.wait_ge(ev_sem, 1)
            sync.dma_start(eg_d[:, :], Ee[:, :]).then_inc(g_sem, 16)
            sync.wait_ge(g_sem, 16)
            sync.dma_start(
                Eg[:, :].rearrange("b (c f) -> b c f", f=FC * FC),
                eg_d[:, :].rearrange("(c b) f -> b c f", b=BL),
            ).then_inc(g_sem, 16)
            # scatter carries Dg[b, c] -> delta[(c,b), slot 0] via DRAM
            sync.wait_ge(p2_sem, 1)
            sync.dma_start(
                dg_d[:, :].rearrange("(c b) f -> b c f", b=BL),
                Dg[:, :].rearrange("b (c f) -> b c f", f=FC),
            ).then_inc(sct_sem, 16)
            sync.wait_ge(sct_sem, 16)
            sync.dma_start(delta[:, 0:FC], dg_d[:, :]).then_inc(sct_sem, 16)
            # outputs
            sync.wait_ge(dv_sem, 1)
            sync.dma_start(ddel_d[:, :],
                           delta[:, FC:(S + 1) * FC]).then_inc(out_sem, 16)

        @block.scalar
        def _(act):
            for b in range(BL):
                ht_load(act, b, 1)
            for b in range(BL):
                act.wait_ge(pe_sem, b + 1)
                act.copy(stage[:, b * T:(b + 1) * T],
                         psum[b][:, :]).then_inc(cp_sem, 1)
                # barrier: wait for the copy's own sem so its tail writes
                # land before the spread DMA reads stage
                act.wait_ge(cp_sem, b + 1)
                # spread hop 1: stage[to, b-block] -> st_d rows {c*8+b},
                # reordered (to, c, t) on the DRAM side
                dst = (st_d[:, :]
                       .rearrange("(c b) (to t) -> b to c t", b=BL, t=S)
                       [b])
                src = (stage[:, b * T:(b + 1) * T]
                       .rearrange("to (c t) -> to c t", t=S))
                act.dma_start(dst, src).then_inc(sp_sem, 16)
            # feats at t=511 for all 8 labels -> host computes final d511
            with nc.allow_non_contiguous_dma(reason="64 gather elems"):
                act.dma_start(
                    f511_d[:, :],
                    stage[:, :].rearrange("p (b t) -> p b t", t=T)
                    [:, :, T - 1],
                ).then_inc(out_sem, 16)
            # spread hop 2: st_d (already in (c,b)-row order) -> feat_sp
            # (same queue as the hop-1 writes, right behind them)
            act.wait_ge(sp_sem, 16 * BL)
            act.dma_start(feat_sp[:, :], st_d[:, :]).then_inc(sp_sem, 16)

        @block.tensor
        def _(te):
            te.wait_ge(wk_sem, 16)
            for b in range(BL):
                # all 3 chunk-DMAs of row b (they complete out of order
                # across DMA engines, so partial counts are not safe)
                te.wait_ge(hs_sems[b], 16 * (KC // 2))
                for kc in range(KC):
                    m = te.matmul(
                        psum[b][:, :],
                        wk[:, kc * LC:(kc + 1) * LC],
                        ht[b][:, kc * T:(kc + 1) * T],
                        start=(kc == 0),
                        stop=(kc == KC - 1),
                    )
                    if kc == KC - 1:
                        m.then_inc(pe_sem, 1)

        @block.gpsimd
        def _(g):
            for b in range(BL):
                ht_load(g, b, 2)

        @block.vector
        def _(v):
            Av = A[:, :].rearrange("p (t to k) -> p t to k", to=FC, k=FC)
            dlt = delta[:, :].rearrange("p (s f) -> p s f", f=FC)

            def ovn(n):
                """Compact level scratch [to, j(n), f, k] in sw16."""
                return (sw16[:, 0:n * FC * FC * FC]
                        .rearrange("p (to j f k) -> p to j f k",
                                   j=n, f=FC, k=FC))

            v.wait_ge(in_sem, 80)
            v.wait_ge(sp_sem, 16 * BL + 16)
            # feat transpose-copy to fp16 [t, to]
            v.tensor_scalar_add(
                feat_tp[:, :].rearrange("p (t to) -> p t to", to=LC),
                feat_sp[:, :].rearrange("p (to t) -> p t to", to=LC), 0.0)
            v.engine_nop()
            # seed carry: D_0 = delta_1 = trans[f,START]+bias[f]+feat_1[f]
            f1 = (feat_sp[0:BL, :].rearrange("p (to t) -> p to t", to=LC)
                  [:, 0:FC, 1:2].rearrange("p f a -> p (f a)"))
            v.tensor_tensor(Dg[:, 0:FC], d7c[:, :], f1, op=ADD)
            v.engine_nop()

            # level 0 (pairs) from the G3 decomposition, all-fp16 operands:
            #   G3[j,to,k] = feat[2j+1,to] + feat[2j,k] + trep7[to,k]
            #   pair_j[to,f] = max_k( G3[j,to,k] + trep7[k,f] )
            ftv = feat_tp[:, :].rearrange("p (t to) -> p t to", to=LC)
            fodd = (ftv[:, 1:S:2, 0:FC].unsqueeze(3)
                    .broadcast_to([128, NP, FC, FC]))
            fevn = (ftv[:, 0:S:2, 0:FC].unsqueeze(2)
                    .broadcast_to([128, NP, FC, FC]))
            f2v = F2[:, :].rearrange("p (j to k) -> p j to k", to=FC, k=FC)
            v.tensor_tensor(f2v, fodd, fevn, op=ADD)
            g3v = G3[:, :].rearrange("p (j to k) -> p j to k", to=FC, k=FC)
            t7h = (trep7h[:, :].rearrange("p (to k) -> p to k", k=FC)
                   .unsqueeze(1).broadcast_to([128, NP, FC, FC]))
            v.tensor_tensor(g3v, f2v, t7h, op=ADD)
            v.engine_nop()
            lo0 = (trep7t[:, :].rearrange("p (f k) -> p f k", k=FC)
                   .unsqueeze(1).broadcast_to([128, NP, FC, FC]))
            ov0 = ovn(NP)
            for to in range(FC):
                hi = (g3v[:, :, to, :].unsqueeze(2)
                      .broadcast_to([128, NP, FC, FC]))
                v.tensor_tensor(ov0[:, to], hi, lo0, op=ADD)
            o3 = (sw16[:, :].rearrange("p (tj f k) -> p tj f k",
                                       f=FC, k=FC))
            v.tensor_reduce(Bp[:, :].rearrange("p (tj f) -> p tj f", f=FC),
                            o3, axis=AXX, op=MAX)
            v.engine_nop()
            v.engine_nop()
            # chunk-0 pair 0 := tropical identity
            v.tensor_scalar_add(
                Bp[0:BL, :].rearrange("p (to j f) -> p to j f",
                                      j=NP, f=FC)[:, :, 0],
                identB[:, :].rearrange("p (to f) -> p to f", f=FC), 0.0)
            v.engine_nop()
            v.engine_nop()

            def copy_t(dstT, srcN, n):
                """dstT[j,x,y] = srcN-product M_j[y,x] (to-major src)."""
                o = dstT[:, :].rearrange("p (j x y) -> p j x y", x=FC, y=FC)
                i = srcN[:, :].rearrange("p (y j x) -> p j x y", j=n, x=FC)
                v.tensor_scalar_add(o, i, 0.0)
                v.engine_nop()
                v.engine_nop()

            copy_t(BpT, Bp, NP)

            # levels 1..4: all-fp16, dual-orientation sources
            for (dstN, dstT, srcN, srcT, n) in [
                    (T2, T2T, Bp, BpT, 8), (T3, T3T, T2, T2T, 4),
                    (T4, T4T, T3, T3T, 2), (Ee, None, T4, T4T, 1)]:
                m = 2 * n
                sv = srcN[:, :].rearrange("p (to j k) -> p j to k",
                                          j=m, k=FC)
                lo = (srcT[:, :].rearrange("p (j f k) -> p j f k",
                                           f=FC, k=FC)[:, 0:m:2])
                ovl = ovn(n)
                for to in range(FC):
                    hi = (sv[:, 1:m:2, to, :].unsqueeze(2)
                          .broadcast_to([128, n, FC, FC]))
                    v.tensor_tensor(ovl[:, to], hi, lo, op=ADD)
                o3n = (sw16[:, 0:n * FC * FC * FC]
                       .rearrange("p (tj f k) -> p tj f k", f=FC, k=FC))
                d2 = dstN[:, :].rearrange("p (tj f) -> p tj f", f=FC)
                v.tensor_reduce(d2, o3n, axis=AXX, op=MAX)
                v.engine_nop()
                v.engine_nop()
                if dstT is not None:
                    copy_t(dstT, dstN, n)
            v.engine_nop().then_inc(ev_sem, 1)

            # A matrices (f32) for the phase-3 fills; overlaps the gather
            fv = (feat_sp[:, :].rearrange("p (to t) -> p t to", to=LC)
                  [:, :, 0:FC].unsqueeze(3).broadcast_to([128, S, FC, FC]))
            tv = (trep7[:, :].rearrange("p (to k) -> p to k", k=FC)
                  .unsqueeze(1).broadcast_to([128, S, FC, FC]))
            v.tensor_tensor(Av, tv, fv, op=ADD)
            v.engine_nop()

            # phase 2: carries D_{c+1} = E_c (x) D_c  (b-partition layout)
            v.wait_ge(g_sem, 32)
            egv = Eg[:, :].rearrange("p (c to k) -> p c to k", to=FC, k=FC)
            s2 = sc2[:, :].rearrange("p (to k) -> p to k", k=FC)
            for c in range(C - 1):
                din = (Dg[:, c * FC:(c + 1) * FC]
                       .rearrange("p (a k) -> p a k", a=1)
                       .broadcast_to([BL, FC, FC]))
                v.tensor_tensor(s2, egv[:, c], din, op=ADD)
                v.tensor_reduce(Dg[:, (c + 1) * FC:(c + 2) * FC], s2,
                                axis=AXX, op=MAX)
                v.engine_nop()
                v.engine_nop()
            v.engine_nop().then_inc(p2_sem, 1)

            # phase 3: re-scan. serial over quads, then two bulk fills.
            v.wait_ge(sct_sem, 32)
            s3 = scw[:, 0:FC * FC].rearrange("p (to k) -> p to k", k=FC)
            t2q = T2[:, :].rearrange("p (to i f) -> p i to f", i=8, f=FC)
            for i in range(8):
                din = (delta[:, 4 * i * FC:(4 * i + 1) * FC]
                       .rearrange("p (a k) -> p a k", a=1)
                       .broadcast_to([128, FC, FC]))
                v.tensor_tensor(s3, t2q[:, i], din, op=ADD)
                v.tensor_reduce(delta[:, (4 * i + 4) * FC:(4 * i + 5) * FC],
                                s3, axis=AXX, op=MAX)
                v.engine_nop()
                v.engine_nop()
            # pairs-fill: local_{4i+1} = B_{2i} (x) local_{4i-1}, i=0..7
            bq = (Bp[:, :].rearrange("p (to j k) -> p j to k",
                                     j=NP, k=FC)[:, 0:NP:2])
            dq = (dlt[:, 0:S:4, :].unsqueeze(2)
                  .broadcast_to([128, 8, FC, FC]))
            oq = (scw[:, 0:8 * FC * FC]
                  .rearrange("p (i to k) -> p i to k", to=FC, k=FC))
            v.tensor_tensor(oq, bq, dq, op=ADD)
            v.tensor_reduce(dlt[:, 2:S:4, :], oq, axis=AXX, op=MAX)
            v.engine_nop()
            v.engine_nop()
            # evens: local_{2j} = A_{2j} (x) local_{2j-1} for all j at once
            ae = (Av[:, 0:S:2, :, :])                          # [p,16,7,7]
            de = (dlt[:, 0:S:2, :].unsqueeze(2)
                  .broadcast_to([128, NP, FC, FC]))
            oe = (scw[:, 0:NP * FC * FC]
                  .rearrange("p (j to k) -> p j to k", to=FC, k=FC))
            v.tensor_tensor(oe, ae, de, op=ADD)
            v.tensor_reduce(dlt[:, 1:S:2, :], oe, axis=AXX, op=MAX)
            v.engine_nop().then_inc(dv_sem, 1)

    return nc


_PROG = None


def _get_prog():
    global _PROG
    if _PROG is None:
        _PROG = build_program()
    return _PROG


def make_in_maps(hidden_states, W, b, transitions):
    import ml_dtypes
    hs = np.asarray(hidden_states, np.float32)
    W = np.asarray(W, np.float32)
    bb = np.asarray(b, np.float32)
    trans = np.asarray(transitions, np.float32)

    Wc = W[:, LAB]                                       # [768, 8]
    wk = np.ascontiguousarray(Wc.reshape(KC, 128, LC).transpose(1, 0, 2)
                              ).reshape(128, KC * LC).astype(ml_dtypes.bfloat16)
    t7 = (trans + bb[:, None])[0:FC, 0:FC]               # [7, 7]
    trep7 = np.ascontiguousarray(
        np.broadcast_to(t7.reshape(1, FC * FC), (128, FC * FC))).astype(
            np.float32)
    trep7h = trep7.astype(np.float16)
    trep7t = np.ascontiguousarray(
        np.broadcast_to(t7.T.reshape(1, FC * FC),
                        (128, FC * FC))).astype(np.float16)
    d7c = np.ascontiguousarray(
        np.broadcast_to((trans[0:FC, START] + bb[0:FC])[None, :],
                        (BL, FC))).astype(np.float32)
    idm = np.where(np.eye(FC, dtype=bool), 0.0, NEG)
    identB = np.ascontiguousarray(
        np.broadcast_to(idm.reshape(1, FC * FC),
                        (BL, FC * FC))).astype(np.float16)

    in_maps = []
    for c in range(NC):
        shard = hs[c * BL:(c + 1) * BL]                 # [8, 512, 768]
        hsT = np.ascontiguousarray(shard.transpose(0, 2, 1)).astype(
            ml_dtypes.bfloat16)                         # [8, 768, 512]
        in_maps.append({"hsT": hsT, "wk": wk, "trep7": trep7,
                        "trep7h": trep7h, "trep7t": trep7t,
                        "d7c": d7c, "identB": identB})
    return in_maps


def decode(ddel_list, f511_list, transitions, bias):
    """ddel [128, 224] f32 per core, f511 [8(to), 8(b)] -> path [64,512]."""
    trans = np.asarray(transitions, np.float32)
    bias = np.asarray(bias, np.float32)
    lab = np.array(LAB, np.int64)
    t8 = trans[LAB][:, 0:FC] + bias[LAB][:, None]        # [8to, 7k]
    delta = np.empty((B, T, FC), np.float32)
    d8 = np.empty((B, LC), np.float32)
    for c in range(NC):
        dd = ddel_list[c].reshape(C, BL, S, FC)          # [(c,b), j, f]
        delta[c * BL:(c + 1) * BL] = (dd.transpose(1, 0, 2, 3)
                                      .reshape(BL, T, FC))
        d510 = delta[c * BL:(c + 1) * BL, T - 2, :]      # [b, 7]
        d8[c * BL:(c + 1) * BL] = ((t8[None, :, :] + d510[:, None, :])
                                   .max(-1) + f511_list[c].T)
    path = np.empty((B, T), np.int32)
    cur = lab[np.argmax(d8, axis=1)]                     # labels, may be 8
    path[:, T - 1] = cur
    for t in range(T - 1, 1, -1):
        cur = np.argmax(trans[cur, 0:FC] + delta[:, t - 1, :], axis=1)
        path[:, t - 1] = cur
    path[:, 0] = START
    return path


def kernel(hidden_states, W, b, transitions):
    in_maps = make_in_maps(hidden_states, W, b, transitions)
    nc = _get_prog()
    res = run_bass_kernel_spmd(nc, in_maps, list(range(NC))).results
    return decode([res[c]["ddel"] for c in range(NC)],
                  [res[c]["f511"] for c in range(NC)], transitions, b)


# revision 47
# speedup vs baseline: 1.0492x; 1.0492x over previous
"""BERT-CRF NER Viterbi decode kernel for Trainium2 (8 NeuronCores).

Strategy (data-parallel over batch, 8 rows/core), raw Bass:
  - host: shard hidden_states [64,512,768] -> 8 x [8,512,768], pre-transpose
    to [8,768,512] and cast to bf16 (halves the dominant HBM read; validated
    ~3e-3 path mismatch, far under the 2e-2 gate). W compact+bf16.
  - device (per core):
      feats = W.T @ hsT per batch row -> PSUM [8,512] (6 K-chunks, bf16 PE,
        inputs streamed in kc-pair chunks over 3 DMA queues: SP/ACT/Pool)
      ACT copies PSUM->SBUF stage; DMA spreads feats to a chunked layout
        feat_sp[p = c*8+b, (to,t_local)]  (C=16 time-chunks of S=32 steps)
      Tree-compose (max,+) pair products per chunk in fp16 (TensorTensor
        runs 2x on packed fp16; products kept in both orientations so every
        operand is unit-stride): level 0 decomposes A = trep + feat into
        G3[j,to,k] = feat[2j+1,to]+feat[2j,k]+trep[to,k] then composes with
        the constant trep^T; levels 1..4 pair up products to the chunk
        product E.  Chunk-0 pair 0 is overwritten with the tropical
        identity (the uniform recurrence starts at t=2 with carry delta_1).
      Carry chain: gather E to [b, c] layout via a DRAM bounce, 15 serial
        matrix-vector steps D_{c+1} = E_c (x) D_c (f32), scatter back.
      Phase 3 re-scan per chunk: 8 serial quad steps + two bulk fills
        (pairs, then evens via f32 A matrices) -> delta_t for all t (f32).
  - host: final-step argmax from f511 dump, psi + backtrace from delta
    (identical argmax semantics to the reference; restricted to from-labels
    0..6 which provably always win).
"""

import numpy as np
from contextlib import ExitStack

import concourse.bass as bass
from concourse import mybir
from concourse.bass_utils import run_bass_kernel_spmd

B, T, H, L = 64, 512, 768, 9
NC = 8              # cores
BL = B // NC        # batch rows per core = 8
KC = H // 128       # 6 contraction chunks
C = 16              # time chunks per sequence
S = T // C          # 32 steps per chunk
NP = S // 2         # 16 pairs per chunk
START = 7
NEG = -10000.0

F32 = mybir.dt.float32
F16 = mybir.dt.float16
BF16 = mybir.dt.bfloat16
ADD = mybir.AluOpType.add
MAX = mybir.AluOpType.max
AXX = mybir.AxisListType.X

LC = 8          # compact 'to' labels: (0..6, 8); START row dropped
FC = 7          # compact 'from' labels: 0..6
LAB = [0, 1, 2, 3, 4, 5, 6, 8]


def build_program():
    nc = bass.Bass("TRN2", target_bir_lowering=False,
                   detect_race_conditions=False)

    hsT_d = nc.dram_tensor("hsT", [BL, H, T], BF16, kind="ExternalInput")
    wk_d = nc.dram_tensor("wk", [128, KC * LC], BF16, kind="ExternalInput")
    trep7_d = nc.dram_tensor("trep7", [128, FC * FC], F32,
                             kind="ExternalInput")
    trep7h_d = nc.dram_tensor("trep7h", [128, FC * FC], F16,
                              kind="ExternalInput")
    trep7t_d = nc.dram_tensor("trep7t", [128, FC * FC], F16,
                              kind="ExternalInput")
    d7c_d = nc.dram_tensor("d7c", [BL, FC], F32, kind="ExternalInput")
    identB_d = nc.dram_tensor("identB", [BL, FC * FC], F16,
                              kind="ExternalInput")
    # bounce buffers for cross-partition regroups
    eg_d = nc.dram_tensor("egb", [128, FC * FC], F16, kind="Internal")
    dg_d = nc.dram_tensor("dgb", [128, FC], F32, kind="Internal")
    st_d = nc.dram_tensor("stb", [128, LC * S], F32, kind="Internal")
    ddel_d = nc.dram_tensor("ddel", [128, S * FC], F32,
                            kind="ExternalOutput")
    f511_d = nc.dram_tensor("f511", [LC, BL], F32, kind="ExternalOutput")

    with ExitStack() as ctx:
        def sb(name, shape, dt=F32):
            return ctx.enter_context(nc.sbuf_tensor(name, shape, dt))
        wk = sb("wk_sb", [128, KC * LC], BF16)
        trep7 = sb("trep7_sb", [128, FC * FC])
        trep7h = sb("trep7h_sb", [128, FC * FC], F16)
        trep7t = sb("trep7t_sb", [128, FC * FC], F16)
        d7c = sb("d7c_sb", [BL, FC])
        identB = sb("identB_sb", [BL, FC * FC], F16)
        ht = [sb(f"ht{i}", [128, KC * T], BF16) for i in range(BL)]
        stage = sb("stage", [LC, BL * T])
        feat_sp = sb("feat_sp", [128, LC * S])
        feat_tp = sb("feat_tp", [128, S * LC], F16)   # [t, to] fp16
        A = sb("A_sb", [128, S * FC * FC])            # f32, fills only
        F2 = sb("F2", [128, NP * FC * FC], F16)
        G3 = sb("G3", [128, NP * FC * FC], F16)
        sw16 = sb("sw16", [128, NP * FC * FC * FC], F16)
        scw = sb("scw", [128, NP * FC * FC])          # f32 fill scratch
        Bp = sb("Bp", [128, NP * FC * FC], F16)       # pair products
        BpT = sb("BpT", [128, NP * FC * FC], F16)
        T2 = sb("T2", [128, 8 * FC * FC], F16)
        T2T = sb("T2T", [128, 8 * FC * FC], F16)
        T3 = sb("T3", [128, 4 * FC * FC], F16)
        T3T = sb("T3T", [128, 4 * FC * FC], F16)
        T4 = sb("T4", [128, 2 * FC * FC], F16)
        T4T = sb("T4T", [128, 2 * FC * FC], F16)
        Ee = sb("Ee", [128, FC * FC], F16)            # chunk product
        Eg = sb("Eg", [BL, C * FC * FC], F16)         # gathered [b, c]
        Dg = sb("Dg", [BL, C * FC])                   # carries [b, c] f32
        sc2 = sb("sc2", [BL, FC * FC])
        delta = sb("delta", [128, (S + 1) * FC])      # slot i = local i-1
        psum = [ctx.enter_context(nc.psum_tensor(f"psum{b}", [LC, T], F32))
                for b in range(BL)]

        in_sem = ctx.enter_context(nc.semaphore("in_sem"))
        wk_sem = ctx.enter_context(nc.semaphore("wk_sem"))
        hs_sems = [ctx.enter_context(nc.semaphore(f"hs_sem{i}"))
                   for i in range(BL)]
        pe_sem = ctx.enter_context(nc.semaphore("pe_sem"))
        cp_sem = ctx.enter_context(nc.semaphore("cp_sem"))
        sp_sem = ctx.enter_context(nc.semaphore("sp_sem"))
        ev_sem = ctx.enter_context(nc.semaphore("ev_sem"))
        g_sem = ctx.enter_context(nc.semaphore("g_sem"))
        p2_sem = ctx.enter_context(nc.semaphore("p2_sem"))
        sct_sem = ctx.enter_context(nc.semaphore("sct_sem"))
        dv_sem = ctx.enter_context(nc.semaphore("dv_sem"))
        p2h_sem = ctx.enter_context(nc.semaphore("p2h_sem"))
        out_sem = ctx.enter_context(nc.semaphore("out_sem"))
        block = ctx.enter_context(nc.Block())

        def ht_load(eng, b, kcp):
            """Load kc-pair chunk kcp of batch row b (PE streams behind)."""
            src = (hsT_d[b, :, :].rearrange("(kc p) t -> p kc t", p=128)
                   [:, 2 * kcp:2 * kcp + 2, :])
            dst = (ht[b][:, :].rearrange("p (kc t) -> p kc t", kc=KC)
                   [:, 2 * kcp:2 * kcp + 2, :])
            eng.dma_start(dst, src).then_inc(hs_sems[b], 16)

        @block.sync
        def _(sync):
            # wk first so the PE can start ASAP; hs chunks striped across
            # the 3 DMA queues (chunk q of each row on queue q) so row b
            # lands ~(b+1) transfer-times in, pipelining the PE perfectly
            sync.dma_start(wk[:, :], wk_d[:, :]).then_inc(wk_sem, 16)
            for b in range(BL):
                ht_load(sync, b, 0)
            sync.dma_start(trep7[:, :], trep7_d[:, :]).then_inc(in_sem, 16)
            sync.dma_start(trep7h[:, :], trep7h_d[:, :]).then_inc(in_sem, 16)
            sync.dma_start(trep7t[:, :], trep7t_d[:, :]).then_inc(in_sem, 16)
            sync.dma_start(d7c[:, :], d7c_d[:, :]).then_inc(in_sem, 16)
            sync.dma_start(identB[:, :], identB_d[:, :]).then_inc(in_sem, 16)
            # gather chunk products E[(c,b)] -> Eg[b, (c,...)] via DRAM
            sync.wait_ge(ev_sem, 1)
            sync.dma_start(eg_d[:, :], Ee[:, :]).then_inc(g_sem, 16)
            sync.wait_ge(g_sem, 16)
            sync.dma_start(
                Eg[:, :].rearrange("b (c f) -> b c f", f=FC * FC),
                eg_d[:, :].rearrange("(c b) f -> b c f", b=BL),
            ).then_inc(g_sem, 16)
            # scatter carries Dg[b, c] -> delta[(c,b), slot 0] via DRAM
            # hop 1 in halves: the first overlaps phase 2's last 7 steps
            sync.wait_ge(p2h_sem, 1)
            sync.dma_start(
                dg_d[:, :].rearrange("(c b) f -> b c f", b=BL)[:, 0:9],
                Dg[:, 0:9 * FC].rearrange("b (c f) -> b c f", f=FC),
            ).then_inc(sct_sem, 16)
            sync.wait_ge(p2_sem, 1)
            sync.dma_start(
                dg_d[:, :].rearrange("(c b) f -> b c f", b=BL)[:, 9:C],
                Dg[:, 9 * FC:].rearrange("b (c f) -> b c f", f=FC),
            ).then_inc(sct_sem, 16)
            sync.wait_ge(sct_sem, 32)
            sync.dma_start(delta[:, 0:FC], dg_d[:, :]).then_inc(sct_sem, 16)
            # outputs
            sync.wait_ge(dv_sem, 1)
            sync.dma_start(ddel_d[:, :],
                           delta[:, FC:(S + 1) * FC]).then_inc(out_sem, 16)

        @block.scalar
        def _(act):
            for b in range(BL):
                ht_load(act, b, 1)
            for b in range(BL):
                act.wait_ge(pe_sem, b + 1)
                act.copy(stage[:, b * T:(b + 1) * T],
                         psum[b][:, :]).then_inc(cp_sem, 1)
                # barrier: wait for the copy's own sem so its tail writes
                # land before the spread DMA reads stage
                act.wait_ge(cp_sem, b + 1)
                # spread hop 1: stage[to, b-block] -> st_d rows {c*8+b},
                # reordered (to, c, t) on the DRAM side
                dst = (st_d[:, :]
                       .rearrange("(c b) (to t) -> b to c t", b=BL, t=S)
                       [b])
                src = (stage[:, b * T:(b + 1) * T]
                       .rearrange("to (c t) -> to c t", t=S))
                act.dma_start(dst, src).then_inc(sp_sem, 16)
            # spread hop 2: st_d (already in (c,b)-row order) -> feat_sp
            # (same queue as the hop-1 writes, right behind them)
            act.wait_ge(sp_sem, 16 * BL)
            act.dma_start(feat_sp[:, :], st_d[:, :]).then_inc(sp_sem, 16)
            # feats at t=511 for all 8 labels -> host computes final d511
            with nc.allow_non_contiguous_dma(reason="64 gather elems"):
                act.dma_start(
                    f511_d[:, :],
                    stage[:, :].rearrange("p (b t) -> p b t", t=T)
                    [:, :, T - 1],
                ).then_inc(out_sem, 16)

        @block.tensor
        def _(te):
            te.wait_ge(wk_sem, 16)
            for b in range(BL):
                # all 3 chunk-DMAs of row b (they complete out of order
                # across DMA engines, so partial counts are not safe)
                te.wait_ge(hs_sems[b], 16 * (KC // 2))
                for kc in range(KC):
                    m = te.matmul(
                        psum[b][:, :],
                        wk[:, kc * LC:(kc + 1) * LC],
                        ht[b][:, kc * T:(kc + 1) * T],
                        start=(kc == 0),
                        stop=(kc == KC - 1),
                    )
                    if kc == KC - 1:
                        m.then_inc(pe_sem, 1)

        @block.gpsimd
        def _(g):
            for b in range(BL):
                ht_load(g, b, 2)

        @block.vector
        def _(v):
            Av = A[:, :].rearrange("p (t to k) -> p t to k", to=FC, k=FC)
            dlt = delta[:, :].rearrange("p (s f) -> p s f", f=FC)

            def ovn(n):
                """Compact level scratch [to, j(n), f, k] in sw16."""
                return (sw16[:, 0:n * FC * FC * FC]
                        .rearrange("p (to j f k) -> p to j f k",
                                   j=n, f=FC, k=FC))

            v.wait_ge(in_sem, 80)
            v.wait_ge(sp_sem, 16 * BL + 16)
            # feat transpose-copy to fp16 [t, to]
            v.tensor_scalar_add(
                feat_tp[:, :].rearrange("p (t to) -> p t to", to=LC),
                feat_sp[:, :].rearrange("p (to t) -> p t to", to=LC), 0.0)
            v.engine_nop()
            # seed carry: D_0 = delta_1 = trans[f,START]+bias[f]+feat_1[f]
            f1 = (feat_sp[0:BL, :].rearrange("p (to t) -> p to t", to=LC)
                  [:, 0:FC, 1:2].rearrange("p f a -> p (f a)"))
            v.tensor_tensor(Dg[:, 0:FC], d7c[:, :], f1, op=ADD)
            v.engine_nop()

            # level 0 (pairs) from the G3 decomposition, all-fp16 operands:
            #   G3[j,to,k] = feat[2j+1,to] + feat[2j,k] + trep7[to,k]
            #   pair_j[to,f] = max_k( G3[j,to,k] + trep7[k,f] )
            ftv = feat_tp[:, :].rearrange("p (t to) -> p t to", to=LC)
            fodd = (ftv[:, 1:S:2, 0:FC].unsqueeze(3)
                    .broadcast_to([128, NP, FC, FC]))
            fevn = (ftv[:, 0:S:2, 0:FC].unsqueeze(2)
                    .broadcast_to([128, NP, FC, FC]))
            f2v = F2[:, :].rearrange("p (j to k) -> p j to k", to=FC, k=FC)
            v.tensor_tensor(f2v, fodd, fevn, op=ADD)
            g3v = G3[:, :].rearrange("p (j to k) -> p j to k", to=FC, k=FC)
            t7h = (trep7h[:, :].rearrange("p (to k) -> p to k", k=FC)
                   .unsqueeze(1).broadcast_to([128, NP, FC, FC]))
            v.tensor_tensor(g3v, f2v, t7h, op=ADD)
            v.engine_nop()
            lo0 = (trep7t[:, :].rearrange("p (f k) -> p f k", k=FC)
                   .unsqueeze(1).broadcast_to([128, NP, FC, FC]))
            ov0 = ovn(NP)
            for to in range(FC):
                hi = (g3v[:, :, to, :].unsqueeze(2)
                      .broadcast_to([128, NP, FC, FC]))
                v.tensor_tensor(ov0[:, to], hi, lo0, op=ADD)
            o3 = (sw16[:, :].rearrange("p (tj f k) -> p tj f k",
                                       f=FC, k=FC))
            v.tensor_reduce(Bp[:, :].rearrange("p (tj f) -> p tj f", f=FC),
                            o3, axis=AXX, op=MAX)
            v.engine_nop()
            v.engine_nop()
            # chunk-0 pair 0 := tropical identity
            v.tensor_scalar_add(
                Bp[0:BL, :].rearrange("p (to j f) -> p to j f",
                                      j=NP, f=FC)[:, :, 0],
                identB[:, :].rearrange("p (to f) -> p to f", f=FC), 0.0)
            v.engine_nop()
            v.engine_nop()

            def copy_t(dstT, srcN, n):
                """dstT[j,x,y] = srcN-product M_j[y,x] (to-major src)."""
                o = dstT[:, :].rearrange("p (j x y) -> p j x y", x=FC, y=FC)
                i = srcN[:, :].rearrange("p (y j x) -> p j x y", j=n, x=FC)
                v.tensor_scalar_add(o, i, 0.0)
                v.engine_nop()
                v.engine_nop()

            copy_t(BpT, Bp, NP)

            # levels 1..4: all-fp16, dual-orientation sources
            for (dstN, dstT, srcN, srcT, n) in [
                    (T2, T2T, Bp, BpT, 8), (T3, T3T, T2, T2T, 4),
                    (T4, T4T, T3, T3T, 2), (Ee, None, T4, T4T, 1)]:
                m = 2 * n
                sv = srcN[:, :].rearrange("p (to j k) -> p j to k",
                                          j=m, k=FC)
                lo = (srcT[:, :].rearrange("p (j f k) -> p j f k",
                                           f=FC, k=FC)[:, 0:m:2])
                ovl = ovn(n)
                for to in range(FC):
                    hi = (sv[:, 1:m:2, to, :].unsqueeze(2)
                          .broadcast_to([128, n, FC, FC]))
                    v.tensor_tensor(ovl[:, to], hi, lo, op=ADD)
                o3n = (sw16[:, 0:n * FC * FC * FC]
                       .rearrange("p (tj f k) -> p tj f k", f=FC, k=FC))
                d2 = dstN[:, :].rearrange("p (tj f) -> p tj f", f=FC)
                v.tensor_reduce(d2, o3n, axis=AXX, op=MAX)
                v.engine_nop()
                v.engine_nop()
                if dstT is not None:
                    copy_t(dstT, dstN, n)
            v.engine_nop().then_inc(ev_sem, 1)

            # A matrices (f32) for the phase-3 fills; overlaps the gather
            fv = (feat_sp[:, :].rearrange("p (to t) -> p t to", to=LC)
                  [:, :, 0:FC].unsqueeze(3).broadcast_to([128, S, FC, FC]))
            tv = (trep7[:, :].rearrange("p (to k) -> p to k", k=FC)
                  .unsqueeze(1).broadcast_to([128, S, FC, FC]))
            v.tensor_tensor(Av, tv, fv, op=ADD)
            v.engine_nop()

            # phase 2: carries D_{c+1} = E_c (x) D_c  (b-partition layout)
            v.wait_ge(g_sem, 32)
            egv = Eg[:, :].rearrange("p (c to k) -> p c to k", to=FC, k=FC)
            s2 = sc2[:, :].rearrange("p (to k) -> p to k", k=FC)
            for c in range(C - 1):
                din = (Dg[:, c * FC:(c + 1) * FC]
                       .rearrange("p (a k) -> p a k", a=1)
                       .broadcast_to([BL, FC, FC]))
                v.tensor_tensor(s2, egv[:, c], din, op=ADD)
                v.tensor_reduce(Dg[:, (c + 1) * FC:(c + 2) * FC], s2,
                                axis=AXX, op=MAX)
                v.engine_nop()
                if c == 7:
                    v.engine_nop().then_inc(p2h_sem, 1)  # D_0..D_8 final
                else:
                    v.engine_nop()
            v.engine_nop().then_inc(p2_sem, 1)

            # phase 3: re-scan. serial over quads, then two bulk fills.
            v.wait_ge(sct_sem, 48)
            s3 = scw[:, 0:FC * FC].rearrange("p (to k) -> p to k", k=FC)
            t2q = T2[:, :].rearrange("p (to i f) -> p i to f", i=8, f=FC)
            for i in range(8):
                din = (delta[:, 4 * i * FC:(4 * i + 1) * FC]
                       .rearrange("p (a k) -> p a k", a=1)
                       .broadcast_to([128, FC, FC]))
                v.tensor_tensor(s3, t2q[:, i], din, op=ADD)
                v.tensor_reduce(delta[:, (4 * i + 4) * FC:(4 * i + 5) * FC],
                                s3, axis=AXX, op=MAX)
                v.engine_nop()
                v.engine_nop()
            # pairs-fill: local_{4i+1} = B_{2i} (x) local_{4i-1}, i=0..7
            bq = (Bp[:, :].rearrange("p (to j k) -> p j to k",
                                     j=NP, k=FC)[:, 0:NP:2])
            dq = (dlt[:, 0:S:4, :].unsqueeze(2)
                  .broadcast_to([128, 8, FC, FC]))
            oq = (scw[:, 0:8 * FC * FC]
                  .rearrange("p (i to k) -> p i to k", to=FC, k=FC))
            v.tensor_tensor(oq, bq, dq, op=ADD)
            v.tensor_reduce(dlt[:, 2:S:4, :], oq, axis=AXX, op=MAX)
            v.engine_nop()
            v.engine_nop()
            # evens: local_{2j} = A_{2j} (x) local_{2j-1} for all j at once
            ae = (Av[:, 0:S:2, :, :])                          # [p,16,7,7]
            de = (dlt[:, 0:S:2, :].unsqueeze(2)
                  .broadcast_to([128, NP, FC, FC]))
            oe = (scw[:, 0:NP * FC * FC]
                  .rearrange("p (j to k) -> p j to k", to=FC, k=FC))
            v.tensor_tensor(oe, ae, de, op=ADD)
            v.tensor_reduce(dlt[:, 1:S:2, :], oe, axis=AXX, op=MAX)
            v.engine_nop().then_inc(dv_sem, 1)

    return nc


_PROG = None


def _get_prog():
    global _PROG
    if _PROG is None:
        _PROG = build_program()
    return _PROG


def make_in_maps(hidden_states, W, b, transitions):
    import ml_dtypes
    hs = np.asarray(hidden_states, np.float32)
    W = np.asarray(W, np.float32)
    bb = np.asarray(b, np.float32)
    trans = np.asarray(transitions, np.float32)

    Wc = W[:, LAB]                                       # [768, 8]
    wk = np.ascontiguousarray(Wc.reshape(KC, 128, LC).transpose(1, 0, 2)
                              ).reshape(128, KC * LC).astype(ml_dtypes.bfloat16)
    t7 = (trans + bb[:, None])[0:FC, 0:FC]               # [7, 7]
    trep7 = np.ascontiguousarray(
        np.broadcast_to(t7.reshape(1, FC * FC), (128, FC * FC))).astype(
            np.float32)
    trep7h = trep7.astype(np.float16)
    trep7t = np.ascontiguousarray(
        np.broadcast_to(t7.T.reshape(1, FC * FC),
                        (128, FC * FC))).astype(np.float16)
    d7c = np.ascontiguousarray(
        np.broadcast_to((trans[0:FC, START] + bb[0:FC])[None, :],
                        (BL, FC))).astype(np.float32)
    idm = np.where(np.eye(FC, dtype=bool), 0.0, NEG)
    identB = np.ascontiguousarray(
        np.broadcast_to(idm.reshape(1, FC * FC),
                        (BL, FC * FC))).astype(np.float16)

    in_maps = []
    for c in range(NC):
        shard = hs[c * BL:(c + 1) * BL]                 # [8, 512, 768]
        hsT = np.ascontiguousarray(shard.transpose(0, 2, 1)).astype(
            ml_dtypes.bfloat16)                         # [8, 768, 512]
        in_maps.append({"hsT": hsT, "wk": wk, "trep7": trep7,
                        "trep7h": trep7h, "trep7t": trep7t,
                        "d7c": d7c, "identB": identB})
    return in_maps


def decode(ddel_list, f511_list, transitions, bias):
    """ddel [128, 224] f32 per core, f511 [8(to), 8(b)] -> path [64,512]."""
    trans = np.asarray(transitions, np.float32)
    bias = np.asarray(bias, np.float32)
    lab = np.array(LAB, np.int64)
    t8 = trans[LAB][:, 0:FC] + bias[LAB][:, None]        # [8to, 7k]
    delta = np.empty((B, T, FC), np.float32)
    d8 = np.empty((B, LC), np.float32)
    for c in range(NC):
        dd = ddel_list[c].reshape(C, BL, S, FC)          # [(c,b), j, f]
        delta[c * BL:(c + 1) * BL] = (dd.transpose(1, 0, 2, 3)
                                      .reshape(BL, T, FC))
        d510 = delta[c * BL:(c + 1) * BL, T - 2, :]      # [b, 7]
        d8[c * BL:(c + 1) * BL] = ((t8[None, :, :] + d510[:, None, :])
                                   .max(-1) + f511_list[c].T)
    path = np.empty((B, T), np.int32)
    cur = lab[np.argmax(d8, axis=1)]                     # labels, may be 8
    path[:, T - 1] = cur
    for t in range(T - 1, 1, -1):
        cur = np.argmax(trans[cur, 0:FC] + delta[:, t - 1, :], axis=1)
        path[:, t - 1] = cur
    path[:, 0] = START
    return path


def kernel(hidden_states, W, b, transitions):
    in_maps = make_in_maps(hidden_states, W, b, transitions)
    nc = _get_prog()
    res = run_bass_kernel_spmd(nc, in_maps, list(range(NC))).results
    return decode([res[c]["ddel"] for c in range(NC)],
                  [res[c]["f511"] for c in range(NC)], transitions, b)


# revision 50
# speedup vs baseline: 1.0823x; 1.0315x over previous
"""BERT-CRF NER Viterbi decode kernel for Trainium2 (8 NeuronCores).

Strategy (data-parallel over batch, 8 rows/core), raw Bass:
  - host: shard hidden_states [64,512,768] -> 8 x [8,512,768], pre-transpose
    to [8,768,512] and cast to bf16 (halves the dominant HBM read; validated
    ~3e-3 path mismatch, far under the 2e-2 gate). W compact+bf16.
  - device (per core):
      feats = W.T @ hsT per batch row -> PSUM [8,512] (6 K-chunks, bf16 PE,
        inputs streamed in kc-pair chunks over 3 DMA queues: SP/ACT/Pool)
      ACT copies PSUM->SBUF stage; DMA spreads feats to a chunked layout
        feat_sp[p = c*8+b, (to,t_local)]  (C=16 time-chunks of S=32 steps)
      Tree-compose (max,+) pair products per chunk in fp16 (TensorTensor
        runs 2x on packed fp16; products kept in both orientations so every
        operand is unit-stride): level 0 decomposes A = trep + feat into
        G3[j,to,k] = feat[2j+1,to]+feat[2j,k]+trep[to,k] then composes with
        the constant trep^T; levels 1..4 pair up products to the chunk
        product E.  Chunk-0 pair 0 is overwritten with the tropical
        identity (the uniform recurrence starts at t=2 with carry delta_1).
      Carry chain: gather E to [b, c] layout via a DRAM bounce, 15 serial
        matrix-vector steps D_{c+1} = E_c (x) D_c (f32), scatter back.
      Phase 3 re-scan per chunk: 8 serial quad steps + two bulk fills
        (pairs, then evens via f32 A matrices) -> delta_t for all t (f32).
  - host: final-step argmax from f511 dump, psi + backtrace from delta
    (identical argmax semantics to the reference; restricted to from-labels
    0..6 which provably always win).
"""

import numpy as np
from contextlib import ExitStack

import concourse.bass as bass
from concourse import mybir
from concourse.bass_utils import run_bass_kernel_spmd

B, T, H, L = 64, 512, 768, 9
NC = 8              # cores
BL = B // NC        # batch rows per core = 8
KC = H // 128       # 6 contraction chunks
C = 16              # time chunks per sequence
S = T // C          # 32 steps per chunk
NP = S // 2         # 16 pairs per chunk
START = 7
NEG = -10000.0

F32 = mybir.dt.float32
F16 = mybir.dt.float16
BF16 = mybir.dt.bfloat16
ADD = mybir.AluOpType.add
MAX = mybir.AluOpType.max
BYP = mybir.AluOpType.bypass
AXX = mybir.AxisListType.X

LC = 8          # compact 'to' labels: (0..6, 8); START row dropped
FC = 7          # compact 'from' labels: 0..6
LAB = [0, 1, 2, 3, 4, 5, 6, 8]


def build_program():
    nc = bass.Bass("TRN2", target_bir_lowering=False,
                   detect_race_conditions=False)

    hsT_d = nc.dram_tensor("hsT", [BL, H, T], BF16, kind="ExternalInput")
    wk_d = nc.dram_tensor("wk", [128, KC * LC], BF16, kind="ExternalInput")
    trep7_d = nc.dram_tensor("trep7", [128, FC * FC], F32,
                             kind="ExternalInput")
    trep7h_d = nc.dram_tensor("trep7h", [128, FC * FC], F16,
                              kind="ExternalInput")
    trep7t_d = nc.dram_tensor("trep7t", [128, FC * FC], F16,
                              kind="ExternalInput")
    d7c_d = nc.dram_tensor("d7c", [BL, FC], F32, kind="ExternalInput")
    identB_d = nc.dram_tensor("identB", [BL, FC * FC], F16,
                              kind="ExternalInput")
    # bounce buffers for cross-partition regroups
    eg_d = nc.dram_tensor("egb", [128, FC * FC], F16, kind="Internal")
    dg_d = nc.dram_tensor("dgb", [128, FC], F32, kind="Internal")
    st_d = nc.dram_tensor("stb", [128, LC * S], F32, kind="Internal")
    ddel_d = nc.dram_tensor("ddel", [128, S * FC], F32,
                            kind="ExternalOutput")
    f511_d = nc.dram_tensor("f511", [LC, BL], F32, kind="ExternalOutput")

    with ExitStack() as ctx:
        def sb(name, shape, dt=F32):
            return ctx.enter_context(nc.sbuf_tensor(name, shape, dt))
        wk = sb("wk_sb", [128, KC * LC], BF16)
        trep7 = sb("trep7_sb", [128, FC * FC])
        trep7h = sb("trep7h_sb", [128, FC * FC], F16)
        trep7t = sb("trep7t_sb", [128, FC * FC], F16)
        d7c = sb("d7c_sb", [BL, FC])
        identB = sb("identB_sb", [BL, FC * FC], F16)
        ht = [sb(f"ht{i}", [128, KC * T], BF16) for i in range(BL)]
        stage = sb("stage", [LC, BL * T])
        feat_sp = sb("feat_sp", [128, LC * S])
        feat_tp = sb("feat_tp", [128, S * LC], F16)   # [t, to] fp16
        A = sb("A_sb", [128, S * FC * FC])            # f32, fills only
        F2 = sb("F2", [128, NP * FC * FC], F16)
        G3 = sb("G3", [128, NP * FC * FC], F16)
        sw16 = sb("sw16", [128, NP * FC * FC * FC], F16)
        mra = sb("mra", [128, NP * FC * FC * 2], F16)
        mrb = sb("mrb", [128, NP * FC * FC * 2], F16)
        scw = sb("scw", [128, NP * FC * FC])          # f32 fill scratch
        Bp = sb("Bp", [128, NP * FC * FC], F16)       # pair products
        BpT = sb("BpT", [128, NP * FC * FC], F16)
        T2 = sb("T2", [128, 8 * FC * FC], F16)
        T2T = sb("T2T", [128, 8 * FC * FC], F16)
        T3 = sb("T3", [128, 4 * FC * FC], F16)
        T3T = sb("T3T", [128, 4 * FC * FC], F16)
        T4 = sb("T4", [128, 2 * FC * FC], F16)
        T4T = sb("T4T", [128, 2 * FC * FC], F16)
        Ee = sb("Ee", [128, FC * FC], F16)            # chunk product
        Eg = sb("Eg", [BL, C * FC * FC], F16)         # gathered [b, c]
        Dg = sb("Dg", [BL, C * FC])                   # carries [b, c] f32
        sc2 = sb("sc2", [BL, FC * FC])
        delta = sb("delta", [128, (S + 1) * FC])      # slot i = local i-1
        psum = [ctx.enter_context(nc.psum_tensor(f"psum{b}", [LC, T], F32))
                for b in range(BL)]

        in_sem = ctx.enter_context(nc.semaphore("in_sem"))
        wk_sem = ctx.enter_context(nc.semaphore("wk_sem"))
        hs_sems = [ctx.enter_context(nc.semaphore(f"hs_sem{i}"))
                   for i in range(BL)]
        pe_sem = ctx.enter_context(nc.semaphore("pe_sem"))
        cp_sem = ctx.enter_context(nc.semaphore("cp_sem"))
        sp_sem = ctx.enter_context(nc.semaphore("sp_sem"))
        ev_sem = ctx.enter_context(nc.semaphore("ev_sem"))
        g_sem = ctx.enter_context(nc.semaphore("g_sem"))
        p2_sem = ctx.enter_context(nc.semaphore("p2_sem"))
        sct_sem = ctx.enter_context(nc.semaphore("sct_sem"))
        dv_sem = ctx.enter_context(nc.semaphore("dv_sem"))
        p2h_sem = ctx.enter_context(nc.semaphore("p2h_sem"))
        out_sem = ctx.enter_context(nc.semaphore("out_sem"))
        block = ctx.enter_context(nc.Block())

        def ht_load(eng, b, kcp):
            """Load kc-pair chunk kcp of batch row b (PE streams behind)."""
            src = (hsT_d[b, :, :].rearrange("(kc p) t -> p kc t", p=128)
                   [:, 2 * kcp:2 * kcp + 2, :])
            dst = (ht[b][:, :].rearrange("p (kc t) -> p kc t", kc=KC)
                   [:, 2 * kcp:2 * kcp + 2, :])
            eng.dma_start(dst, src).then_inc(hs_sems[b], 16)

        @block.sync
        def _(sync):
            # wk first so the PE can start ASAP; hs chunks striped across
            # the 3 DMA queues (chunk q of each row on queue q) so row b
            # lands ~(b+1) transfer-times in, pipelining the PE perfectly
            sync.dma_start(wk[:, :], wk_d[:, :]).then_inc(wk_sem, 16)
            for b in range(BL):
                ht_load(sync, b, 0)
            sync.dma_start(trep7[:, :], trep7_d[:, :]).then_inc(in_sem, 16)
            sync.dma_start(trep7h[:, :], trep7h_d[:, :]).then_inc(in_sem, 16)
            sync.dma_start(trep7t[:, :], trep7t_d[:, :]).then_inc(in_sem, 16)
            sync.dma_start(d7c[:, :], d7c_d[:, :]).then_inc(in_sem, 16)
            sync.dma_start(identB[:, :], identB_d[:, :]).then_inc(in_sem, 16)
            # gather chunk products E[(c,b)] -> Eg[b, (c,...)] via DRAM
            sync.wait_ge(ev_sem, 1)
            sync.dma_start(eg_d[:, :], Ee[:, :]).then_inc(g_sem, 16)
            sync.wait_ge(g_sem, 16)
            sync.dma_start(
                Eg[:, :].rearrange("b (c f) -> b c f", f=FC * FC),
                eg_d[:, :].rearrange("(c b) f -> b c f", b=BL),
            ).then_inc(g_sem, 16)
            # scatter carries Dg[b, c] -> delta[(c,b), slot 0] via DRAM
            # hop 1 in halves: the first overlaps phase 2's last 7 steps
            sync.wait_ge(p2h_sem, 1)
            sync.dma_start(
                dg_d[:, :].rearrange("(c b) f -> b c f", b=BL)[:, 0:9],
                Dg[:, 0:9 * FC].rearrange("b (c f) -> b c f", f=FC),
            ).then_inc(sct_sem, 16)
            sync.wait_ge(p2_sem, 1)
            sync.dma_start(
                dg_d[:, :].rearrange("(c b) f -> b c f", b=BL)[:, 9:C],
                Dg[:, 9 * FC:].rearrange("b (c f) -> b c f", f=FC),
            ).then_inc(sct_sem, 16)
            sync.wait_ge(sct_sem, 32)
            sync.dma_start(delta[:, 0:FC], dg_d[:, :]).then_inc(sct_sem, 16)
            # outputs
            sync.wait_ge(dv_sem, 1)
            sync.dma_start(ddel_d[:, :],
                           delta[:, FC:(S + 1) * FC]).then_inc(out_sem, 16)

        @block.scalar
        def _(act):
            for b in range(BL):
                ht_load(act, b, 1)
            for b in range(BL):
                act.wait_ge(pe_sem, b + 1)
                act.copy(stage[:, b * T:(b + 1) * T],
                         psum[b][:, :]).then_inc(cp_sem, 1)
                # barrier: wait for the copy's own sem so its tail writes
                # land before the spread DMA reads stage
                act.wait_ge(cp_sem, b + 1)
                # spread hop 1: stage[to, b-block] -> st_d rows {c*8+b},
                # reordered (to, c, t) on the DRAM side
                dst = (st_d[:, :]
                       .rearrange("(c b) (to t) -> b to c t", b=BL, t=S)
                       [b])
                src = (stage[:, b * T:(b + 1) * T]
                       .rearrange("to (c t) -> to c t", t=S))
                act.dma_start(dst, src).then_inc(sp_sem, 16)
            # spread hop 2: st_d (already in (c,b)-row order) -> feat_sp
            # (same queue as the hop-1 writes, right behind them)
            act.wait_ge(sp_sem, 16 * BL)
            act.dma_start(feat_sp[:, :], st_d[:, :]).then_inc(sp_sem, 16)
            # feats at t=511 for all 8 labels -> host computes final d511
            with nc.allow_non_contiguous_dma(reason="64 gather elems"):
                act.dma_start(
                    f511_d[:, :],
                    stage[:, :].rearrange("p (b t) -> p b t", t=T)
                    [:, :, T - 1],
                ).then_inc(out_sem, 16)

        @block.tensor
        def _(te):
            te.wait_ge(wk_sem, 16)
            for b in range(BL):
                # all 3 chunk-DMAs of row b (they complete out of order
                # across DMA engines, so partial counts are not safe)
                te.wait_ge(hs_sems[b], 16 * (KC // 2))
                for kc in range(KC):
                    m = te.matmul(
                        psum[b][:, :],
                        wk[:, kc * LC:(kc + 1) * LC],
                        ht[b][:, kc * T:(kc + 1) * T],
                        start=(kc == 0),
                        stop=(kc == KC - 1),
                    )
                    if kc == KC - 1:
                        m.then_inc(pe_sem, 1)

        @block.gpsimd
        def _(g):
            for b in range(BL):
                ht_load(g, b, 2)

        @block.vector
        def _(v):
            Av = A[:, :].rearrange("p (t to k) -> p t to k", to=FC, k=FC)
            dlt = delta[:, :].rearrange("p (s f) -> p s f", f=FC)

            def ovn(n):
                """Compact level scratch [to, j(n), f, k] in sw16."""
                return (sw16[:, 0:n * FC * FC * FC]
                        .rearrange("p (to j f k) -> p to j f k",
                                   j=n, f=FC, k=FC))

            def maxred(dst, n):
                """dst[tjf] = max over k(7) of sw16[tjf, k] via a TT-max
                tree with overlapping covers (TTs run 2x on fp16; the
                TensorReduce has no fast mode).  Covers: a = max(k02,k13),
                b = max(a, k45), c = max(b, k56) -> c0={0,2,4,5},
                c1={1,3,5,6}; then max(c0 | k6-side, c1)."""
                w = FC * FC * n
                o3k = (sw16[:, 0:w * FC]
                       .rearrange("p (tjf k) -> p tjf k", k=FC))
                av = mra[:, 0:2 * w].rearrange("p (tjf k) -> p tjf k", k=2)
                bv = mrb[:, 0:2 * w].rearrange("p (tjf k) -> p tjf k", k=2)
                v.tensor_tensor(av, o3k[:, :, 0:2], o3k[:, :, 2:4], op=MAX)
                v.tensor_tensor(bv, av, o3k[:, :, 4:6], op=MAX)
                v.tensor_tensor(av, bv, o3k[:, :, 5:7], op=MAX)
                v.tensor_tensor(dst, av[:, :, 0], av[:, :, 1], op=MAX)
                v.engine_nop()
                v.engine_nop()

            v.wait_ge(in_sem, 80)
            v.wait_ge(sp_sem, 16 * BL + 16)
            # feat transpose-copy to fp16 [t, to]
            v.tensor_scalar_add(
                feat_tp[:, :].rearrange("p (t to) -> p t to", to=LC),
                feat_sp[:, :].rearrange("p (to t) -> p t to", to=LC), 0.0)
            v.engine_nop()
            # seed carry: D_0 = delta_1 = trans[f,START]+bias[f]+feat_1[f]
            f1 = (feat_sp[0:BL, :].rearrange("p (to t) -> p to t", to=LC)
                  [:, 0:FC, 1:2].rearrange("p f a -> p (f a)"))
            v.tensor_tensor(Dg[:, 0:FC], d7c[:, :], f1, op=ADD)
            v.engine_nop()

            # level 0 (pairs) from the G3 decomposition, all-fp16 operands:
            #   G3[j,to,k] = feat[2j+1,to] + feat[2j,k] + trep7[to,k]
            #   pair_j[to,f] = max_k( G3[j,to,k] + trep7[k,f] )
            ftv = feat_tp[:, :].rearrange("p (t to) -> p t to", to=LC)
            fodd = (ftv[:, 1:S:2, 0:FC].unsqueeze(3)
                    .broadcast_to([128, NP, FC, FC]))
            fevn = (ftv[:, 0:S:2, 0:FC].unsqueeze(2)
                    .broadcast_to([128, NP, FC, FC]))
            f2v = F2[:, :].rearrange("p (j to k) -> p j to k", to=FC, k=FC)
            v.tensor_tensor(f2v, fodd, fevn, op=ADD)
            g3v = G3[:, :].rearrange("p (j to k) -> p j to k", to=FC, k=FC)
            t7h = (trep7h[:, :].rearrange("p (to k) -> p to k", k=FC)
                   .unsqueeze(1).broadcast_to([128, NP, FC, FC]))
            v.tensor_tensor(g3v, f2v, t7h, op=ADD)
            v.engine_nop()
            lo0 = (trep7t[:, :].rearrange("p (f k) -> p f k", k=FC)
                   .unsqueeze(1).broadcast_to([128, NP, FC, FC]))
            ov0 = ovn(NP)
            for to in range(FC):
                hi = (g3v[:, :, to, :].unsqueeze(2)
                      .broadcast_to([128, NP, FC, FC]))
                v.tensor_tensor(ov0[:, to], hi, lo0, op=ADD)
            maxred(Bp[:, :], NP)
            # chunk-0 pair 0 := tropical identity
            v.tensor_scalar_add(
                Bp[0:BL, :].rearrange("p (to j f) -> p to j f",
                                      j=NP, f=FC)[:, :, 0],
                identB[:, :].rearrange("p (to f) -> p to f", f=FC), 0.0)
            v.engine_nop()
            v.engine_nop()

            def copy_t(dstT, srcN, n):
                """dstT[j,x,y] = srcN-product M_j[y,x] (to-major src)."""
                o = dstT[:, :].rearrange("p (j x y) -> p j x y", x=FC, y=FC)
                i = srcN[:, :].rearrange("p (y j x) -> p j x y", j=n, x=FC)
                v.tensor_scalar_add(o, i, 0.0)
                v.engine_nop()
                v.engine_nop()

            copy_t(BpT, Bp, NP)

            # levels 1..4: all-fp16, dual-orientation sources
            for (dstN, dstT, srcN, srcT, n) in [
                    (T2, T2T, Bp, BpT, 8), (T3, T3T, T2, T2T, 4),
                    (T4, T4T, T3, T3T, 2), (Ee, None, T4, T4T, 1)]:
                m = 2 * n
                sv = srcN[:, :].rearrange("p (to j k) -> p j to k",
                                          j=m, k=FC)
                lo = (srcT[:, :].rearrange("p (j f k) -> p j f k",
                                           f=FC, k=FC)[:, 0:m:2])
                ovl = ovn(n)
                for to in range(FC):
                    hi = (sv[:, 1:m:2, to, :].unsqueeze(2)
                          .broadcast_to([128, n, FC, FC]))
                    v.tensor_tensor(ovl[:, to], hi, lo, op=ADD)
                maxred(dstN[:, :], n)
                if dstT is not None:
                    copy_t(dstT, dstN, n)
            v.engine_nop().then_inc(ev_sem, 1)

            # A matrices (f32) for the phase-3 fills; overlaps the gather
            fv = (feat_sp[:, :].rearrange("p (to t) -> p t to", to=LC)
                  [:, :, 0:FC].unsqueeze(3).broadcast_to([128, S, FC, FC]))
            tv = (trep7[:, :].rearrange("p (to k) -> p to k", k=FC)
                  .unsqueeze(1).broadcast_to([128, S, FC, FC]))
            v.tensor_tensor(Av, tv, fv, op=ADD)
            v.engine_nop()

            # phase 2: carries D_{c+1} = E_c (x) D_c  (b-partition layout)
            v.wait_ge(g_sem, 32)
            egv = Eg[:, :].rearrange("p (c to k) -> p c to k", to=FC, k=FC)
            s2 = sc2[:, :].rearrange("p (to k) -> p to k", k=FC)
            for c in range(C - 1):
                din = (Dg[:, c * FC:(c + 1) * FC]
                       .rearrange("p (a k) -> p a k", a=1)
                       .broadcast_to([BL, FC, FC]))
                v.tensor_tensor(s2, egv[:, c], din, op=ADD)
                v.tensor_reduce(Dg[:, (c + 1) * FC:(c + 2) * FC], s2,
                                axis=AXX, op=MAX)
                v.engine_nop()
                if c == 7:
                    v.engine_nop().then_inc(p2h_sem, 1)  # D_0..D_8 final
                else:
                    v.engine_nop()
            v.engine_nop().then_inc(p2_sem, 1)

            # phase 3: re-scan. serial over quads, then two bulk fills.
            v.wait_ge(sct_sem, 48)
            s3 = scw[:, 0:FC * FC].rearrange("p (to k) -> p to k", k=FC)
            t2q = T2[:, :].rearrange("p (to i f) -> p i to f", i=8, f=FC)
            for i in range(8):
                din = (delta[:, 4 * i * FC:(4 * i + 1) * FC]
                       .rearrange("p (a k) -> p a k", a=1)
                       .broadcast_to([128, FC, FC]))
                v.tensor_tensor(s3, t2q[:, i], din, op=ADD)
                v.tensor_reduce(delta[:, (4 * i + 4) * FC:(4 * i + 5) * FC],
                                s3, axis=AXX, op=MAX)
                v.engine_nop()
                v.engine_nop()
            # pairs-fill: local_{4i+1} = B_{2i} (x) local_{4i-1}, i=0..7
            bq = (Bp[:, :].rearrange("p (to j k) -> p j to k",
                                     j=NP, k=FC)[:, 0:NP:2])
            dq = (dlt[:, 0:S:4, :].unsqueeze(2)
                  .broadcast_to([128, 8, FC, FC]))
            oq = (scw[:, 0:8 * FC * FC]
                  .rearrange("p (i to k) -> p i to k", to=FC, k=FC))
            v.tensor_tensor(oq, bq, dq, op=ADD)
            v.tensor_reduce(dlt[:, 2:S:4, :], oq, axis=AXX, op=MAX)
            v.engine_nop()
            v.engine_nop()
            # evens: local_{2j} = A_{2j} (x) local_{2j-1} for all j at once
            ae = (Av[:, 0:S:2, :, :])                          # [p,16,7,7]
            de = (dlt[:, 0:S:2, :].unsqueeze(2)
                  .broadcast_to([128, NP, FC, FC]))
            oe = (scw[:, 0:NP * FC * FC]
                  .rearrange("p (j to k) -> p j to k", to=FC, k=FC))
            v.tensor_tensor(oe, ae, de, op=ADD)
            v.tensor_reduce(dlt[:, 1:S:2, :], oe, axis=AXX, op=MAX)
            v.engine_nop().then_inc(dv_sem, 1)

    return nc


_PROG = None


def _get_prog():
    global _PROG
    if _PROG is None:
        _PROG = build_program()
    return _PROG


def make_in_maps(hidden_states, W, b, transitions):
    import ml_dtypes
    hs = np.asarray(hidden_states, np.float32)
    W = np.asarray(W, np.float32)
    bb = np.asarray(b, np.float32)
    trans = np.asarray(transitions, np.float32)

    Wc = W[:, LAB]                                       # [768, 8]
    wk = np.ascontiguousarray(Wc.reshape(KC, 128, LC).transpose(1, 0, 2)
                              ).reshape(128, KC * LC).astype(ml_dtypes.bfloat16)
    t7 = (trans + bb[:, None])[0:FC, 0:FC]               # [7, 7]
    trep7 = np.ascontiguousarray(
        np.broadcast_to(t7.reshape(1, FC * FC), (128, FC * FC))).astype(
            np.float32)
    trep7h = trep7.astype(np.float16)
    trep7t = np.ascontiguousarray(
        np.broadcast_to(t7.T.reshape(1, FC * FC),
                        (128, FC * FC))).astype(np.float16)
    d7c = np.ascontiguousarray(
        np.broadcast_to((trans[0:FC, START] + bb[0:FC])[None, :],
                        (BL, FC))).astype(np.float32)
    idm = np.where(np.eye(FC, dtype=bool), 0.0, NEG)
    identB = np.ascontiguousarray(
        np.broadcast_to(idm.reshape(1, FC * FC),
                        (BL, FC * FC))).astype(np.float16)

    in_maps = []
    for c in range(NC):
        shard = hs[c * BL:(c + 1) * BL]                 # [8, 512, 768]
        hsT = np.ascontiguousarray(shard.transpose(0, 2, 1)).astype(
            ml_dtypes.bfloat16)                         # [8, 768, 512]
        in_maps.append({"hsT": hsT, "wk": wk, "trep7": trep7,
                        "trep7h": trep7h, "trep7t": trep7t,
                        "d7c": d7c, "identB": identB})
    return in_maps


def decode(ddel_list, f511_list, transitions, bias):
    """ddel [128, 224] f32 per core, f511 [8(to), 8(b)] -> path [64,512]."""
    trans = np.asarray(transitions, np.float32)
    bias = np.asarray(bias, np.float32)
    lab = np.array(LAB, np.int64)
    t8 = trans[LAB][:, 0:FC] + bias[LAB][:, None]        # [8to, 7k]
    delta = np.empty((B, T, FC), np.float32)
    d8 = np.empty((B, LC), np.float32)
    for c in range(NC):
        dd = ddel_list[c].reshape(C, BL, S, FC)          # [(c,b), j, f]
        delta[c * BL:(c + 1) * BL] = (dd.transpose(1, 0, 2, 3)
                                      .reshape(BL, T, FC))
        d510 = delta[c * BL:(c + 1) * BL, T - 2, :]      # [b, 7]
        d8[c * BL:(c + 1) * BL] = ((t8[None, :, :] + d510[:, None, :])
                                   .max(-1) + f511_list[c].T)
    path = np.empty((B, T), np.int32)
    cur = lab[np.argmax(d8, axis=1)]                     # labels, may be 8
    path[:, T - 1] = cur
    for t in range(T - 1, 1, -1):
        cur = np.argmax(trans[cur, 0:FC] + delta[:, t - 1, :], axis=1)
        path[:, t - 1] = cur
    path[:, 0] = START
    return path


def kernel(hidden_states, W, b, transitions):
    in_maps = make_in_maps(hidden_states, W, b, transitions)
    nc = _get_prog()
    res = run_bass_kernel_spmd(nc, in_maps, list(range(NC))).results
    return decode([res[c]["ddel"] for c in range(NC)],
                  [res[c]["f511"] for c in range(NC)], transitions, b)


# revision 51
# speedup vs baseline: 1.0831x; 1.0007x over previous
"""BERT-CRF NER Viterbi decode kernel for Trainium2 (8 NeuronCores).

Strategy (data-parallel over batch, 8 rows/core), raw Bass:
  - host: shard hidden_states [64,512,768] -> 8 x [8,512,768], pre-transpose
    to [8,768,512] and cast to bf16 (halves the dominant HBM read; validated
    ~3e-3 path mismatch, far under the 2e-2 gate). W compact+bf16.
  - device (per core):
      feats = W.T @ hsT per batch row -> PSUM [8,512] (6 K-chunks, bf16 PE,
        inputs streamed in kc-pair chunks over 3 DMA queues: SP/ACT/Pool)
      ACT copies PSUM->SBUF stage; DMA spreads feats to a chunked layout
        feat_sp[p = c*8+b, (to,t_local)]  (C=16 time-chunks of S=32 steps)
      Tree-compose (max,+) pair products per chunk in fp16 (TensorTensor
        runs 2x on packed fp16; products kept in both orientations so every
        operand is unit-stride): level 0 decomposes A = trep + feat into
        G3[j,to,k] = feat[2j+1,to]+feat[2j,k]+trep[to,k] then composes with
        the constant trep^T; levels 1..4 pair up products to the chunk
        product E.  Chunk-0 pair 0 is overwritten with the tropical
        identity (the uniform recurrence starts at t=2 with carry delta_1).
      Carry chain: gather E to [b, c] layout via a DRAM bounce, 15 serial
        matrix-vector steps D_{c+1} = E_c (x) D_c (f32), scatter back.
      Phase 3 re-scan per chunk: 8 serial quad steps + two bulk fills
        (pairs, then evens via f32 A matrices) -> delta_t for all t (f32).
  - host: final-step argmax from f511 dump, psi + backtrace from delta
    (identical argmax semantics to the reference; restricted to from-labels
    0..6 which provably always win).
"""

import numpy as np
from contextlib import ExitStack

import concourse.bass as bass
from concourse import mybir
from concourse.bass_utils import run_bass_kernel_spmd

B, T, H, L = 64, 512, 768, 9
NC = 8              # cores
BL = B // NC        # batch rows per core = 8
KC = H // 128       # 6 contraction chunks
C = 16              # time chunks per sequence
S = T // C          # 32 steps per chunk
NP = S // 2         # 16 pairs per chunk
START = 7
NEG = -10000.0

F32 = mybir.dt.float32
F16 = mybir.dt.float16
BF16 = mybir.dt.bfloat16
ADD = mybir.AluOpType.add
MAX = mybir.AluOpType.max
BYP = mybir.AluOpType.bypass
AXX = mybir.AxisListType.X

LC = 8          # compact 'to' labels: (0..6, 8); START row dropped
FC = 7          # compact 'from' labels: 0..6
LAB = [0, 1, 2, 3, 4, 5, 6, 8]


def build_program():
    nc = bass.Bass("TRN2", target_bir_lowering=False,
                   detect_race_conditions=False)

    hsT_d = nc.dram_tensor("hsT", [BL, H, T], BF16, kind="ExternalInput")
    wk_d = nc.dram_tensor("wk", [128, KC * LC], BF16, kind="ExternalInput")
    trep7_d = nc.dram_tensor("trep7", [128, FC * FC], F32,
                             kind="ExternalInput")
    trep7h_d = nc.dram_tensor("trep7h", [128, FC * FC], F16,
                              kind="ExternalInput")
    trep7t_d = nc.dram_tensor("trep7t", [128, FC * FC], F16,
                              kind="ExternalInput")
    d7c_d = nc.dram_tensor("d7c", [BL, FC], F32, kind="ExternalInput")
    identB_d = nc.dram_tensor("identB", [BL, FC * FC], F16,
                              kind="ExternalInput")
    # bounce buffers for cross-partition regroups
    eg_d = nc.dram_tensor("egb", [128, FC * FC], F16, kind="Internal")
    dg_d = nc.dram_tensor("dgb", [128, FC], F32, kind="Internal")
    st_d = nc.dram_tensor("stb", [128, LC * S], F32, kind="Internal")
    ddel_d = nc.dram_tensor("ddel", [128, S * FC], F32,
                            kind="ExternalOutput")
    f511_d = nc.dram_tensor("f511", [LC, BL], F32, kind="ExternalOutput")

    with ExitStack() as ctx:
        def sb(name, shape, dt=F32):
            return ctx.enter_context(nc.sbuf_tensor(name, shape, dt))
        wk = sb("wk_sb", [128, KC * LC], BF16)
        trep7 = sb("trep7_sb", [128, FC * FC])
        trep7h = sb("trep7h_sb", [128, FC * FC], F16)
        trep7t = sb("trep7t_sb", [128, FC * FC], F16)
        d7c = sb("d7c_sb", [BL, FC])
        identB = sb("identB_sb", [BL, FC * FC], F16)
        ht = [sb(f"ht{i}", [128, KC * T], BF16) for i in range(BL)]
        stage = sb("stage", [LC, BL * T])
        feat_sp = sb("feat_sp", [128, LC * S])
        feat_tp = sb("feat_tp", [128, S * LC], F16)   # [t, to] fp16
        A = sb("A_sb", [128, S * FC * FC])            # f32, fills only
        F2 = sb("F2", [128, NP * FC * FC], F16)
        G3 = sb("G3", [128, NP * FC * FC], F16)
        sw16 = sb("sw16", [128, NP * FC * FC * FC], F16)
        mra = sb("mra", [128, NP * FC * FC * 2], F16)
        mrb = sb("mrb", [128, NP * FC * FC * 2], F16)
        scw = sb("scw", [128, NP * FC * FC])          # f32 fill scratch
        Bp = sb("Bp", [128, NP * FC * FC], F16)       # pair products
        BpT = sb("BpT", [128, NP * FC * FC], F16)
        T2 = sb("T2", [128, 8 * FC * FC], F16)
        T2T = sb("T2T", [128, 8 * FC * FC], F16)
        T3 = sb("T3", [128, 4 * FC * FC], F16)
        T3T = sb("T3T", [128, 4 * FC * FC], F16)
        T4 = sb("T4", [128, 2 * FC * FC], F16)
        T4T = sb("T4T", [128, 2 * FC * FC], F16)
        Ee = sb("Ee", [128, FC * FC], F16)            # chunk product
        Eg = sb("Eg", [BL, C * FC * FC], F16)         # gathered [b, c]
        Dg = sb("Dg", [BL, C * FC])                   # carries [b, c] f32
        sc2 = sb("sc2", [BL, FC * FC])
        delta = sb("delta", [128, (S + 1) * FC])      # slot i = local i-1
        psum = [ctx.enter_context(nc.psum_tensor(f"psum{b}", [LC, T], F32))
                for b in range(BL)]

        in_sem = ctx.enter_context(nc.semaphore("in_sem"))
        wk_sem = ctx.enter_context(nc.semaphore("wk_sem"))
        hs_sems = [ctx.enter_context(nc.semaphore(f"hs_sem{i}"))
                   for i in range(BL)]
        pe_sem = ctx.enter_context(nc.semaphore("pe_sem"))
        cp_sem = ctx.enter_context(nc.semaphore("cp_sem"))
        sp_sem = ctx.enter_context(nc.semaphore("sp_sem"))
        ev_sem = ctx.enter_context(nc.semaphore("ev_sem"))
        g_sem = ctx.enter_context(nc.semaphore("g_sem"))
        gh_sem = ctx.enter_context(nc.semaphore("gh_sem"))
        p2_sem = ctx.enter_context(nc.semaphore("p2_sem"))
        sct_sem = ctx.enter_context(nc.semaphore("sct_sem"))
        dv_sem = ctx.enter_context(nc.semaphore("dv_sem"))
        p2h_sem = ctx.enter_context(nc.semaphore("p2h_sem"))
        out_sem = ctx.enter_context(nc.semaphore("out_sem"))
        block = ctx.enter_context(nc.Block())

        def ht_load(eng, b, kcp):
            """Load kc-pair chunk kcp of batch row b (PE streams behind)."""
            src = (hsT_d[b, :, :].rearrange("(kc p) t -> p kc t", p=128)
                   [:, 2 * kcp:2 * kcp + 2, :])
            dst = (ht[b][:, :].rearrange("p (kc t) -> p kc t", kc=KC)
                   [:, 2 * kcp:2 * kcp + 2, :])
            eng.dma_start(dst, src).then_inc(hs_sems[b], 16)

        @block.sync
        def _(sync):
            # wk first so the PE can start ASAP; hs chunks striped across
            # the 3 DMA queues (chunk q of each row on queue q) so row b
            # lands ~(b+1) transfer-times in, pipelining the PE perfectly
            sync.dma_start(wk[:, :], wk_d[:, :]).then_inc(wk_sem, 16)
            for b in range(BL):
                ht_load(sync, b, 0)
            sync.dma_start(trep7[:, :], trep7_d[:, :]).then_inc(in_sem, 16)
            sync.dma_start(trep7h[:, :], trep7h_d[:, :]).then_inc(in_sem, 16)
            sync.dma_start(trep7t[:, :], trep7t_d[:, :]).then_inc(in_sem, 16)
            sync.dma_start(d7c[:, :], d7c_d[:, :]).then_inc(in_sem, 16)
            sync.dma_start(identB[:, :], identB_d[:, :]).then_inc(in_sem, 16)
            # gather chunk products E[(c,b)] -> Eg[b, (c,...)] via DRAM,
            # in halves so phase 2 starts after the first 8 chunks land
            sync.wait_ge(ev_sem, 1)
            sync.dma_start(eg_d[0:64, :], Ee[0:64, :]).then_inc(g_sem, 16)
            sync.dma_start(eg_d[64:128, :],
                           Ee[64:128, :]).then_inc(gh_sem, 16)
            sync.wait_ge(g_sem, 16)
            sync.dma_start(
                Eg[:, 0:8 * FC * FC].rearrange("b (c f) -> b c f",
                                               f=FC * FC),
                eg_d[0:64, :].rearrange("(c b) f -> b c f", b=BL),
            ).then_inc(g_sem, 16)
            sync.wait_ge(gh_sem, 16)
            sync.dma_start(
                Eg[:, 8 * FC * FC:].rearrange("b (c f) -> b c f",
                                              f=FC * FC),
                eg_d[64:128, :].rearrange("(c b) f -> b c f", b=BL),
            ).then_inc(gh_sem, 16)
            # scatter carries Dg[b, c] -> delta[(c,b), slot 0] via DRAM
            # hop 1 in halves: the first overlaps phase 2's last 7 steps
            sync.wait_ge(p2h_sem, 1)
            sync.dma_start(
                dg_d[:, :].rearrange("(c b) f -> b c f", b=BL)[:, 0:9],
                Dg[:, 0:9 * FC].rearrange("b (c f) -> b c f", f=FC),
            ).then_inc(sct_sem, 16)
            sync.wait_ge(p2_sem, 1)
            sync.dma_start(
                dg_d[:, :].rearrange("(c b) f -> b c f", b=BL)[:, 9:C],
                Dg[:, 9 * FC:].rearrange("b (c f) -> b c f", f=FC),
            ).then_inc(sct_sem, 16)
            sync.wait_ge(sct_sem, 32)
            sync.dma_start(delta[:, 0:FC], dg_d[:, :]).then_inc(sct_sem, 16)
            # outputs
            sync.wait_ge(dv_sem, 1)
            sync.dma_start(ddel_d[:, :],
                           delta[:, FC:(S + 1) * FC]).then_inc(out_sem, 16)

        @block.scalar
        def _(act):
            for b in range(BL):
                ht_load(act, b, 1)
            for b in range(BL):
                act.wait_ge(pe_sem, b + 1)
                act.copy(stage[:, b * T:(b + 1) * T],
                         psum[b][:, :]).then_inc(cp_sem, 1)
                # barrier: wait for the copy's own sem so its tail writes
                # land before the spread DMA reads stage
                act.wait_ge(cp_sem, b + 1)
                # spread hop 1: stage[to, b-block] -> st_d rows {c*8+b},
                # reordered (to, c, t) on the DRAM side
                dst = (st_d[:, :]
                       .rearrange("(c b) (to t) -> b to c t", b=BL, t=S)
                       [b])
                src = (stage[:, b * T:(b + 1) * T]
                       .rearrange("to (c t) -> to c t", t=S))
                act.dma_start(dst, src).then_inc(sp_sem, 16)
            # spread hop 2: st_d (already in (c,b)-row order) -> feat_sp
            # (same queue as the hop-1 writes, right behind them)
            act.wait_ge(sp_sem, 16 * BL)
            act.dma_start(feat_sp[:, :], st_d[:, :]).then_inc(sp_sem, 16)
            # feats at t=511 for all 8 labels -> host computes final d511
            with nc.allow_non_contiguous_dma(reason="64 gather elems"):
                act.dma_start(
                    f511_d[:, :],
                    stage[:, :].rearrange("p (b t) -> p b t", t=T)
                    [:, :, T - 1],
                ).then_inc(out_sem, 16)

        @block.tensor
        def _(te):
            te.wait_ge(wk_sem, 16)
            for b in range(BL):
                # all 3 chunk-DMAs of row b (they complete out of order
                # across DMA engines, so partial counts are not safe)
                te.wait_ge(hs_sems[b], 16 * (KC // 2))
                for kc in range(KC):
                    m = te.matmul(
                        psum[b][:, :],
                        wk[:, kc * LC:(kc + 1) * LC],
                        ht[b][:, kc * T:(kc + 1) * T],
                        start=(kc == 0),
                        stop=(kc == KC - 1),
                    )
                    if kc == KC - 1:
                        m.then_inc(pe_sem, 1)

        @block.gpsimd
        def _(g):
            for b in range(BL):
                ht_load(g, b, 2)

        @block.vector
        def _(v):
            Av = A[:, :].rearrange("p (t to k) -> p t to k", to=FC, k=FC)
            dlt = delta[:, :].rearrange("p (s f) -> p s f", f=FC)

            def ovn(n):
                """Compact level scratch [to, j(n), f, k] in sw16."""
                return (sw16[:, 0:n * FC * FC * FC]
                        .rearrange("p (to j f k) -> p to j f k",
                                   j=n, f=FC, k=FC))

            def maxred(dst, n):
                """dst[tjf] = max over k(7) of sw16[tjf, k] via a TT-max
                tree with overlapping covers (TTs run 2x on fp16; the
                TensorReduce has no fast mode).  Covers: a = max(k02,k13),
                b = max(a, k45), c = max(b, k56) -> c0={0,2,4,5},
                c1={1,3,5,6}; then max(c0 | k6-side, c1)."""
                w = FC * FC * n
                o3k = (sw16[:, 0:w * FC]
                       .rearrange("p (tjf k) -> p tjf k", k=FC))
                av = mra[:, 0:2 * w].rearrange("p (tjf k) -> p tjf k", k=2)
                bv = mrb[:, 0:2 * w].rearrange("p (tjf k) -> p tjf k", k=2)
                v.tensor_tensor(av, o3k[:, :, 0:2], o3k[:, :, 2:4], op=MAX)
                v.tensor_tensor(bv, av, o3k[:, :, 4:6], op=MAX)
                v.tensor_tensor(av, bv, o3k[:, :, 5:7], op=MAX)
                v.tensor_tensor(dst, av[:, :, 0], av[:, :, 1], op=MAX)
                v.engine_nop()
                v.engine_nop()

            v.wait_ge(in_sem, 80)
            v.wait_ge(sp_sem, 16 * BL + 16)
            # feat transpose-copy to fp16 [t, to]
            v.tensor_scalar_add(
                feat_tp[:, :].rearrange("p (t to) -> p t to", to=LC),
                feat_sp[:, :].rearrange("p (to t) -> p t to", to=LC), 0.0)
            v.engine_nop()
            # seed carry: D_0 = delta_1 = trans[f,START]+bias[f]+feat_1[f]
            f1 = (feat_sp[0:BL, :].rearrange("p (to t) -> p to t", to=LC)
                  [:, 0:FC, 1:2].rearrange("p f a -> p (f a)"))
            v.tensor_tensor(Dg[:, 0:FC], d7c[:, :], f1, op=ADD)
            v.engine_nop()

            # level 0 (pairs) from the G3 decomposition, all-fp16 operands:
            #   G3[j,to,k] = feat[2j+1,to] + feat[2j,k] + trep7[to,k]
            #   pair_j[to,f] = max_k( G3[j,to,k] + trep7[k,f] )
            ftv = feat_tp[:, :].rearrange("p (t to) -> p t to", to=LC)
            fodd = (ftv[:, 1:S:2, 0:FC].unsqueeze(3)
                    .broadcast_to([128, NP, FC, FC]))
            fevn = (ftv[:, 0:S:2, 0:FC].unsqueeze(2)
                    .broadcast_to([128, NP, FC, FC]))
            f2v = F2[:, :].rearrange("p (j to k) -> p j to k", to=FC, k=FC)
            v.tensor_tensor(f2v, fodd, fevn, op=ADD)
            g3v = G3[:, :].rearrange("p (j to k) -> p j to k", to=FC, k=FC)
            t7h = (trep7h[:, :].rearrange("p (to k) -> p to k", k=FC)
                   .unsqueeze(1).broadcast_to([128, NP, FC, FC]))
            v.tensor_tensor(g3v, f2v, t7h, op=ADD)
            v.engine_nop()
            lo0 = (trep7t[:, :].rearrange("p (f k) -> p f k", k=FC)
                   .unsqueeze(1).broadcast_to([128, NP, FC, FC]))
            ov0 = ovn(NP)
            for to in range(FC):
                hi = (g3v[:, :, to, :].unsqueeze(2)
                      .broadcast_to([128, NP, FC, FC]))
                v.tensor_tensor(ov0[:, to], hi, lo0, op=ADD)
            maxred(Bp[:, :], NP)
            # chunk-0 pair 0 := tropical identity
            v.tensor_scalar_add(
                Bp[0:BL, :].rearrange("p (to j f) -> p to j f",
                                      j=NP, f=FC)[:, :, 0],
                identB[:, :].rearrange("p (to f) -> p to f", f=FC), 0.0)
            v.engine_nop()
            v.engine_nop()

            def copy_t(dstT, srcN, n):
                """dstT[j,x,y] = srcN-product M_j[y,x] (to-major src)."""
                o = dstT[:, :].rearrange("p (j x y) -> p j x y", x=FC, y=FC)
                i = srcN[:, :].rearrange("p (y j x) -> p j x y", j=n, x=FC)
                v.tensor_scalar_add(o, i, 0.0)
                v.engine_nop()
                v.engine_nop()

            copy_t(BpT, Bp, NP)

            # levels 1..4: all-fp16, dual-orientation sources
            for (dstN, dstT, srcN, srcT, n) in [
                    (T2, T2T, Bp, BpT, 8), (T3, T3T, T2, T2T, 4),
                    (T4, T4T, T3, T3T, 2), (Ee, None, T4, T4T, 1)]:
                m = 2 * n
                sv = srcN[:, :].rearrange("p (to j k) -> p j to k",
                                          j=m, k=FC)
                lo = (srcT[:, :].rearrange("p (j f k) -> p j f k",
                                           f=FC, k=FC)[:, 0:m:2])
                ovl = ovn(n)
                for to in range(FC):
                    hi = (sv[:, 1:m:2, to, :].unsqueeze(2)
                          .broadcast_to([128, n, FC, FC]))
                    v.tensor_tensor(ovl[:, to], hi, lo, op=ADD)
                maxred(dstN[:, :], n)
                if dstT is not None:
                    copy_t(dstT, dstN, n)
            v.engine_nop().then_inc(ev_sem, 1)

            # A matrices (f32) for the phase-3 fills; overlaps the gather
            fv = (feat_sp[:, :].rearrange("p (to t) -> p t to", to=LC)
                  [:, :, 0:FC].unsqueeze(3).broadcast_to([128, S, FC, FC]))
            tv = (trep7[:, :].rearrange("p (to k) -> p to k", k=FC)
                  .unsqueeze(1).broadcast_to([128, S, FC, FC]))
            v.tensor_tensor(Av, tv, fv, op=ADD)
            v.engine_nop()

            # phase 2: carries D_{c+1} = E_c (x) D_c  (b-partition layout)
            v.wait_ge(g_sem, 32)
            egv = Eg[:, :].rearrange("p (c to k) -> p c to k", to=FC, k=FC)
            s2 = sc2[:, :].rearrange("p (to k) -> p to k", k=FC)
            for c in range(C - 1):
                if c == 8:
                    v.wait_ge(gh_sem, 32)
                din = (Dg[:, c * FC:(c + 1) * FC]
                       .rearrange("p (a k) -> p a k", a=1)
                       .broadcast_to([BL, FC, FC]))
                v.tensor_tensor(s2, egv[:, c], din, op=ADD)
                v.tensor_reduce(Dg[:, (c + 1) * FC:(c + 2) * FC], s2,
                                axis=AXX, op=MAX)
                v.engine_nop()
                if c == 7:
                    v.engine_nop().then_inc(p2h_sem, 1)  # D_0..D_8 final
                else:
                    v.engine_nop()
            v.engine_nop().then_inc(p2_sem, 1)

            # phase 3: re-scan. 4 serial oct steps, then three bulk fills.
            v.wait_ge(sct_sem, 48)
            s3 = scw[:, 0:FC * FC].rearrange("p (to k) -> p to k", k=FC)
            t3q = T3[:, :].rearrange("p (to i f) -> p i to f", i=4, f=FC)
            for i in range(4):
                din = (delta[:, 8 * i * FC:(8 * i + 1) * FC]
                       .rearrange("p (a k) -> p a k", a=1)
                       .broadcast_to([128, FC, FC]))
                v.tensor_tensor(s3, t3q[:, i], din, op=ADD)
                v.tensor_reduce(delta[:, (8 * i + 8) * FC:(8 * i + 9) * FC],
                                s3, axis=AXX, op=MAX)
                v.engine_nop()
                v.engine_nop()
            # quad-fill: local_{8i+3} = T2_{2i} (x) local_{8i-1}, i=0..3
            t2e = (T2[:, :].rearrange("p (to i k) -> p i to k",
                                      i=8, k=FC)[:, 0:8:2])
            d8q = (dlt[:, 0:S + 1:8, :][:, 0:4].unsqueeze(2)
                   .broadcast_to([128, 4, FC, FC]))
            o4q = (scw[:, 0:4 * FC * FC]
                   .rearrange("p (i to k) -> p i to k", to=FC, k=FC))
            v.tensor_tensor(o4q, t2e, d8q, op=ADD)
            v.tensor_reduce(dlt[:, 4:S:8, :], o4q, axis=AXX, op=MAX)
            v.engine_nop()
            v.engine_nop()
            # pairs-fill: local_{4i+1} = B_{2i} (x) local_{4i-1}, i=0..7
            bq = (Bp[:, :].rearrange("p (to j k) -> p j to k",
                                     j=NP, k=FC)[:, 0:NP:2])
            dq = (dlt[:, 0:S:4, :].unsqueeze(2)
                  .broadcast_to([128, 8, FC, FC]))
            oq = (scw[:, 0:8 * FC * FC]
                  .rearrange("p (i to k) -> p i to k", to=FC, k=FC))
            v.tensor_tensor(oq, bq, dq, op=ADD)
            v.tensor_reduce(dlt[:, 2:S:4, :], oq, axis=AXX, op=MAX)
            v.engine_nop()
            v.engine_nop()
            # evens: local_{2j} = A_{2j} (x) local_{2j-1} for all j at once
            ae = (Av[:, 0:S:2, :, :])                          # [p,16,7,7]
            de = (dlt[:, 0:S:2, :].unsqueeze(2)
                  .broadcast_to([128, NP, FC, FC]))
            oe = (scw[:, 0:NP * FC * FC]
                  .rearrange("p (j to k) -> p j to k", to=FC, k=FC))
            v.tensor_tensor(oe, ae, de, op=ADD)
            v.tensor_reduce(dlt[:, 1:S:2, :], oe, axis=AXX, op=MAX)
            v.engine_nop().then_inc(dv_sem, 1)

    return nc


_PROG = None


def _get_prog():
    global _PROG
    if _PROG is None:
        _PROG = build_program()
    return _PROG


def make_in_maps(hidden_states, W, b, transitions):
    import ml_dtypes
    hs = np.asarray(hidden_states, np.float32)
    W = np.asarray(W, np.float32)
    bb = np.asarray(b, np.float32)
    trans = np.asarray(transitions, np.float32)

    Wc = W[:, LAB]                                       # [768, 8]
    wk = np.ascontiguousarray(Wc.reshape(KC, 128, LC).transpose(1, 0, 2)
                              ).reshape(128, KC * LC).astype(ml_dtypes.bfloat16)
    t7 = (trans + bb[:, None])[0:FC, 0:FC]               # [7, 7]
    trep7 = np.ascontiguousarray(
        np.broadcast_to(t7.reshape(1, FC * FC), (128, FC * FC))).astype(
            np.float32)
    trep7h = trep7.astype(np.float16)
    trep7t = np.ascontiguousarray(
        np.broadcast_to(t7.T.reshape(1, FC * FC),
                        (128, FC * FC))).astype(np.float16)
    d7c = np.ascontiguousarray(
        np.broadcast_to((trans[0:FC, START] + bb[0:FC])[None, :],
                        (BL, FC))).astype(np.float32)
    idm = np.where(np.eye(FC, dtype=bool), 0.0, NEG)
    identB = np.ascontiguousarray(
        np.broadcast_to(idm.reshape(1, FC * FC),
                        (BL, FC * FC))).astype(np.float16)

    in_maps = []
    for c in range(NC):
        shard = hs[c * BL:(c + 1) * BL]                 # [8, 512, 768]
        hsT = np.ascontiguousarray(shard.transpose(0, 2, 1)).astype(
            ml_dtypes.bfloat16)                         # [8, 768, 512]
        in_maps.append({"hsT": hsT, "wk": wk, "trep7": trep7,
                        "trep7h": trep7h, "trep7t": trep7t,
                        "d7c": d7c, "identB": identB})
    return in_maps


def decode(ddel_list, f511_list, transitions, bias):
    """ddel [128, 224] f32 per core, f511 [8(to), 8(b)] -> path [64,512]."""
    trans = np.asarray(transitions, np.float32)
    bias = np.asarray(bias, np.float32)
    lab = np.array(LAB, np.int64)
    t8 = trans[LAB][:, 0:FC] + bias[LAB][:, None]        # [8to, 7k]
    delta = np.empty((B, T, FC), np.float32)
    d8 = np.empty((B, LC), np.float32)
    for c in range(NC):
        dd = ddel_list[c].reshape(C, BL, S, FC)          # [(c,b), j, f]
        delta[c * BL:(c + 1) * BL] = (dd.transpose(1, 0, 2, 3)
                                      .reshape(BL, T, FC))
        d510 = delta[c * BL:(c + 1) * BL, T - 2, :]      # [b, 7]
        d8[c * BL:(c + 1) * BL] = ((t8[None, :, :] + d510[:, None, :])
                                   .max(-1) + f511_list[c].T)
    path = np.empty((B, T), np.int32)
    cur = lab[np.argmax(d8, axis=1)]                     # labels, may be 8
    path[:, T - 1] = cur
    for t in range(T - 1, 1, -1):
        cur = np.argmax(trans[cur, 0:FC] + delta[:, t - 1, :], axis=1)
        path[:, t - 1] = cur
    path[:, 0] = START
    return path


def kernel(hidden_states, W, b, transitions):
    in_maps = make_in_maps(hidden_states, W, b, transitions)
    nc = _get_prog()
    res = run_bass_kernel_spmd(nc, in_maps, list(range(NC))).results
    return decode([res[c]["ddel"] for c in range(NC)],
                  [res[c]["f511"] for c in range(NC)], transitions, b)


# revision 53
# speedup vs baseline: 1.0907x; 1.0070x over previous
"""BERT-CRF NER Viterbi decode kernel for Trainium2 (8 NeuronCores).

Strategy (data-parallel over batch, 8 rows/core), raw Bass:
  - host: shard hidden_states [64,512,768] -> 8 x [8,512,768], pre-transpose
    to [8,768,512] and cast to bf16 (halves the dominant HBM read; validated
    ~3e-3 path mismatch, far under the 2e-2 gate). W compact+bf16.
  - device (per core):
      feats = W.T @ hsT per batch row -> PSUM [8,512] (6 K-chunks, bf16 PE,
        inputs streamed in kc-pair chunks over 3 DMA queues: SP/ACT/Pool)
      ACT copies PSUM->SBUF stage; DMA spreads feats to a chunked layout
        feat_sp[p = c*8+b, (to,t_local)]  (C=16 time-chunks of S=32 steps)
      Tree-compose (max,+) pair products per chunk in fp16 (TensorTensor
        runs 2x on packed fp16; products kept in both orientations so every
        operand is unit-stride): level 0 decomposes A = trep + feat into
        G3[j,to,k] = feat[2j+1,to]+feat[2j,k]+trep[to,k] then composes with
        the constant trep^T; levels 1..4 pair up products to the chunk
        product E.  Chunk-0 pair 0 is overwritten with the tropical
        identity (the uniform recurrence starts at t=2 with carry delta_1).
      Carry chain: gather E to [b, c] layout via a DRAM bounce, 15 serial
        matrix-vector steps D_{c+1} = E_c (x) D_c (f32), scatter back.
      Phase 3 re-scan per chunk: 8 serial quad steps + two bulk fills
        (pairs, then evens via f32 A matrices) -> delta_t for all t (f32).
  - host: final-step argmax from f511 dump, psi + backtrace from delta
    (identical argmax semantics to the reference; restricted to from-labels
    0..6 which provably always win).
"""

import numpy as np
from contextlib import ExitStack

import concourse.bass as bass
from concourse import mybir
from concourse.bass_utils import run_bass_kernel_spmd

B, T, H, L = 64, 512, 768, 9
NC = 8              # cores
BL = B // NC        # batch rows per core = 8
KC = H // 128       # 6 contraction chunks
C = 16              # time chunks per sequence
S = T // C          # 32 steps per chunk
NP = S // 2         # 16 pairs per chunk
START = 7
NEG = -10000.0

F32 = mybir.dt.float32
F16 = mybir.dt.float16
BF16 = mybir.dt.bfloat16
ADD = mybir.AluOpType.add
MAX = mybir.AluOpType.max
BYP = mybir.AluOpType.bypass
AXX = mybir.AxisListType.X

LC = 8          # compact 'to' labels: (0..6, 8); START row dropped
FC = 7          # compact 'from' labels: 0..6
LAB = [0, 1, 2, 3, 4, 5, 6, 8]


def build_program():
    nc = bass.Bass("TRN2", target_bir_lowering=False,
                   detect_race_conditions=False)

    hsT_d = nc.dram_tensor("hsT", [BL, H, T], BF16, kind="ExternalInput")
    wk_d = nc.dram_tensor("wk", [128, KC * LC], BF16, kind="ExternalInput")
    trep7_d = nc.dram_tensor("trep7", [128, FC * FC], F32,
                             kind="ExternalInput")
    trep7h_d = nc.dram_tensor("trep7h", [128, FC * FC], F16,
                              kind="ExternalInput")
    trep7t_d = nc.dram_tensor("trep7t", [128, FC * FC], F16,
                              kind="ExternalInput")
    d7c_d = nc.dram_tensor("d7c", [BL, FC], F32, kind="ExternalInput")
    identB_d = nc.dram_tensor("identB", [BL, FC * FC], F16,
                              kind="ExternalInput")
    # bounce buffers for cross-partition regroups
    eg_d = nc.dram_tensor("egb", [128, FC * FC], F16, kind="Internal")
    dg_d = nc.dram_tensor("dgb", [128, FC], F32, kind="Internal")
    st_d = nc.dram_tensor("stb", [128, LC * S], F32, kind="Internal")
    ddel_d = nc.dram_tensor("ddel", [128, S * FC], F32,
                            kind="ExternalOutput")
    f511_d = nc.dram_tensor("f511", [LC, BL], F32, kind="ExternalOutput")

    with ExitStack() as ctx:
        def sb(name, shape, dt=F32):
            return ctx.enter_context(nc.sbuf_tensor(name, shape, dt))
        wk = sb("wk_sb", [128, KC * LC], BF16)
        trep7 = sb("trep7_sb", [128, FC * FC])
        trep7h = sb("trep7h_sb", [128, FC * FC], F16)
        trep7t = sb("trep7t_sb", [128, FC * FC], F16)
        d7c = sb("d7c_sb", [BL, FC])
        identB = sb("identB_sb", [BL, FC * FC], F16)
        ht = [sb(f"ht{i}", [128, KC * T], BF16) for i in range(BL)]
        stage = sb("stage", [LC, BL * T])
        feat_sp = sb("feat_sp", [128, LC * S])
        feat_tp = sb("feat_tp", [128, S * LC], F16)   # [t, to] fp16
        A = sb("A_sb", [128, S * FC * FC])            # f32, fills only
        F2 = sb("F2", [128, NP * FC * FC], F16)
        G3 = sb("G3", [128, NP * FC * FC], F16)
        sw16 = sb("sw16", [128, NP * FC * FC * FC], F16)
        mra = sb("mra", [128, NP * FC * FC * 2], F16)
        mrb = sb("mrb", [128, NP * FC * FC * 2], F16)
        scw = sb("scw", [128, NP * FC * FC])          # f32 fill scratch
        Bp = sb("Bp", [128, NP * FC * FC], F16)       # pair products
        BpT = sb("BpT", [128, NP * FC * FC], F16)
        T2 = sb("T2", [128, 8 * FC * FC], F16)
        T2T = sb("T2T", [128, 8 * FC * FC], F16)
        T3 = sb("T3", [128, 4 * FC * FC], F16)
        T3T = sb("T3T", [128, 4 * FC * FC], F16)
        T4 = sb("T4", [128, 2 * FC * FC], F16)
        T4T = sb("T4T", [128, 2 * FC * FC], F16)
        Ee = sb("Ee", [128, FC * FC], F16)            # chunk product
        Eg = sb("Eg", [BL, C * FC * FC], F16)         # gathered [b, c]
        Dg = sb("Dg", [BL, C * FC])                   # carries [b, c] f32
        sc2 = sb("sc2", [BL, FC * FC])
        delta = sb("delta", [128, (S + 1) * FC])      # slot i = local i-1
        psum = [ctx.enter_context(nc.psum_tensor(f"psum{b}", [LC, T], F32))
                for b in range(BL)]

        in_sem = ctx.enter_context(nc.semaphore("in_sem"))
        wk_sem = ctx.enter_context(nc.semaphore("wk_sem"))
        hs_sems = [ctx.enter_context(nc.semaphore(f"hs_sem{i}"))
                   for i in range(BL)]
        pe_sem = ctx.enter_context(nc.semaphore("pe_sem"))
        cp_sem = ctx.enter_context(nc.semaphore("cp_sem"))
        sp_sem = ctx.enter_context(nc.semaphore("sp_sem"))
        ev_sem = ctx.enter_context(nc.semaphore("ev_sem"))
        g_sem = ctx.enter_context(nc.semaphore("g_sem"))
        gh_sem = ctx.enter_context(nc.semaphore("gh_sem"))
        p2_sem = ctx.enter_context(nc.semaphore("p2_sem"))
        sct_sem = ctx.enter_context(nc.semaphore("sct_sem"))
        dv_sem = ctx.enter_context(nc.semaphore("dv_sem"))
        p2h_sem = ctx.enter_context(nc.semaphore("p2h_sem"))
        out_sem = ctx.enter_context(nc.semaphore("out_sem"))
        block = ctx.enter_context(nc.Block())

        def ht_load(eng, b, kcp):
            """Load kc-pair chunk kcp of batch row b (PE streams behind)."""
            src = (hsT_d[b, :, :].rearrange("(kc p) t -> p kc t", p=128)
                   [:, 2 * kcp:2 * kcp + 2, :])
            dst = (ht[b][:, :].rearrange("p (kc t) -> p kc t", kc=KC)
                   [:, 2 * kcp:2 * kcp + 2, :])
            eng.dma_start(dst, src).then_inc(hs_sems[b], 16)

        @block.sync
        def _(sync):
            # wk first so the PE can start ASAP; hs chunks striped across
            # the 3 DMA queues (chunk q of each row on queue q) so row b
            # lands ~(b+1) transfer-times in, pipelining the PE perfectly
            sync.dma_start(wk[:, :], wk_d[:, :]).then_inc(wk_sem, 16)
            for b in range(BL):
                ht_load(sync, b, 0)
            sync.dma_start(trep7[:, :], trep7_d[:, :]).then_inc(in_sem, 16)
            sync.dma_start(trep7h[:, :], trep7h_d[:, :]).then_inc(in_sem, 16)
            sync.dma_start(trep7t[:, :], trep7t_d[:, :]).then_inc(in_sem, 16)
            sync.dma_start(d7c[:, :], d7c_d[:, :]).then_inc(in_sem, 16)
            sync.dma_start(identB[:, :], identB_d[:, :]).then_inc(in_sem, 16)
            # gather chunk products E[(c,b)] -> Eg[b, (c,...)] via DRAM,
            # in halves so phase 2 starts after the first 8 chunks land
            sync.wait_ge(ev_sem, 1)
            sync.dma_start(eg_d[0:64, :], Ee[0:64, :]).then_inc(g_sem, 16)
            sync.dma_start(eg_d[64:128, :],
                           Ee[64:128, :]).then_inc(gh_sem, 16)
            sync.wait_ge(g_sem, 16)
            sync.dma_start(
                Eg[:, 0:8 * FC * FC].rearrange("b (c f) -> b c f",
                                               f=FC * FC),
                eg_d[0:64, :].rearrange("(c b) f -> b c f", b=BL),
            ).then_inc(g_sem, 16)
            sync.wait_ge(gh_sem, 16)
            sync.dma_start(
                Eg[:, 8 * FC * FC:].rearrange("b (c f) -> b c f",
                                              f=FC * FC),
                eg_d[64:128, :].rearrange("(c b) f -> b c f", b=BL),
            ).then_inc(gh_sem, 16)
            # scatter carries Dg[b, c] -> delta[(c,b), slot 0] via DRAM
            # hop 1 in halves: the first overlaps phase 2's last 7 steps
            sync.wait_ge(p2h_sem, 1)
            sync.dma_start(
                dg_d[:, :].rearrange("(c b) f -> b c f", b=BL)[:, 0:9],
                Dg[:, 0:9 * FC].rearrange("b (c f) -> b c f", f=FC),
            ).then_inc(sct_sem, 16)
            sync.wait_ge(p2_sem, 1)
            sync.dma_start(
                dg_d[:, :].rearrange("(c b) f -> b c f", b=BL)[:, 9:C],
                Dg[:, 9 * FC:].rearrange("b (c f) -> b c f", f=FC),
            ).then_inc(sct_sem, 16)
            sync.wait_ge(sct_sem, 32)
            sync.dma_start(delta[:, 0:FC], dg_d[:, :]).then_inc(sct_sem, 16)
            # outputs
            sync.wait_ge(dv_sem, 1)
            sync.dma_start(ddel_d[:, :],
                           delta[:, FC:(S + 1) * FC]).then_inc(out_sem, 16)

        @block.scalar
        def _(act):
            for b in range(BL):
                ht_load(act, b, 1)
            for b in range(BL):
                act.wait_ge(pe_sem, b + 1)
                act.copy(stage[:, b * T:(b + 1) * T],
                         psum[b][:, :]).then_inc(cp_sem, 1)
                # barrier: wait for the copy's own sem so its tail writes
                # land before the spread DMA reads stage
                act.wait_ge(cp_sem, b + 1)
                # spread hop 1: stage[to, b-block] -> st_d rows {c*8+b},
                # reordered (to, c, t) on the DRAM side
                dst = (st_d[:, :]
                       .rearrange("(c b) (to t) -> b to c t", b=BL, t=S)
                       [b])
                src = (stage[:, b * T:(b + 1) * T]
                       .rearrange("to (c t) -> to c t", t=S))
                act.dma_start(dst, src).then_inc(sp_sem, 16)
            # spread hop 2: st_d (already in (c,b)-row order) -> feat_sp
            # (same queue as the hop-1 writes, right behind them)
            act.wait_ge(sp_sem, 16 * BL)
            act.dma_start(feat_sp[:, :], st_d[:, :]).then_inc(sp_sem, 16)
            # feats at t=511 for all 8 labels -> host computes final d511
            with nc.allow_non_contiguous_dma(reason="64 gather elems"):
                act.dma_start(
                    f511_d[:, :],
                    stage[:, :].rearrange("p (b t) -> p b t", t=T)
                    [:, :, T - 1],
                ).then_inc(out_sem, 16)

        @block.tensor
        def _(te):
            te.wait_ge(wk_sem, 16)
            for b in range(BL):
                # all 3 chunk-DMAs of row b (they complete out of order
                # across DMA engines, so partial counts are not safe)
                te.wait_ge(hs_sems[b], 16 * (KC // 2))
                for kc in range(KC):
                    m = te.matmul(
                        psum[b][:, :],
                        wk[:, kc * LC:(kc + 1) * LC],
                        ht[b][:, kc * T:(kc + 1) * T],
                        start=(kc == 0),
                        stop=(kc == KC - 1),
                    )
                    if kc == KC - 1:
                        m.then_inc(pe_sem, 1)

        @block.gpsimd
        def _(g):
            for b in range(BL):
                ht_load(g, b, 2)

        @block.vector
        def _(v):
            Av = A[:, :].rearrange("p (t to k) -> p t to k", to=FC, k=FC)
            dlt = delta[:, :].rearrange("p (s f) -> p s f", f=FC)

            def ovn(n):
                """Compact level scratch [to, j(n), f, k] in sw16."""
                return (sw16[:, 0:n * FC * FC * FC]
                        .rearrange("p (to j f k) -> p to j f k",
                                   j=n, f=FC, k=FC))

            def maxred(dst, n):
                """dst[tjf] = max over k(7) of sw16[tjf, k] via a TT-max
                tree with overlapping covers (TTs run 2x on fp16; the
                TensorReduce has no fast mode).  Covers: a = max(k02,k13),
                b = max(a, k45), c = max(b, k56) -> c0={0,2,4,5},
                c1={1,3,5,6}; then max(c0 | k6-side, c1)."""
                w = FC * FC * n
                o3k = (sw16[:, 0:w * FC]
                       .rearrange("p (tjf k) -> p tjf k", k=FC))
                av = mra[:, 0:2 * w].rearrange("p (tjf k) -> p tjf k", k=2)
                bv = mrb[:, 0:2 * w].rearrange("p (tjf k) -> p tjf k", k=2)
                v.tensor_tensor(av, o3k[:, :, 0:2], o3k[:, :, 2:4], op=MAX)
                v.tensor_tensor(bv, av, o3k[:, :, 4:6], op=MAX)
                v.tensor_tensor(av, bv, o3k[:, :, 5:7], op=MAX)
                v.tensor_tensor(dst, av[:, :, 0], av[:, :, 1], op=MAX)
                v.engine_nop()
                v.engine_nop()

            v.wait_ge(in_sem, 80)
            v.wait_ge(sp_sem, 16 * BL + 16)
            # feat transpose-copy to fp16 [t, to]
            v.tensor_scalar_add(
                feat_tp[:, :].rearrange("p (t to) -> p t to", to=LC),
                feat_sp[:, :].rearrange("p (to t) -> p t to", to=LC), 0.0)
            v.engine_nop()
            # seed carry: D_0 = delta_1 = trans[f,START]+bias[f]+feat_1[f]
            f1 = (feat_sp[0:BL, :].rearrange("p (to t) -> p to t", to=LC)
                  [:, 0:FC, 1:2].rearrange("p f a -> p (f a)"))
            v.tensor_tensor(Dg[:, 0:FC], d7c[:, :], f1, op=ADD)
            v.engine_nop()

            # level 0 (pairs) from the G3 decomposition, all-fp16 operands:
            #   G3[j,to,k] = feat[2j+1,to] + feat[2j,k] + trep7[to,k]
            #   pair_j[to,f] = max_k( G3[j,to,k] + trep7[k,f] )
            ftv = feat_tp[:, :].rearrange("p (t to) -> p t to", to=LC)
            fodd = (ftv[:, 1:S:2, 0:FC].unsqueeze(3)
                    .broadcast_to([128, NP, FC, FC]))
            fevn = (ftv[:, 0:S:2, 0:FC].unsqueeze(2)
                    .broadcast_to([128, NP, FC, FC]))
            f2v = F2[:, :].rearrange("p (j to k) -> p j to k", to=FC, k=FC)
            v.tensor_tensor(f2v, fodd, fevn, op=ADD)
            g3v = G3[:, :].rearrange("p (j to k) -> p j to k", to=FC, k=FC)
            t7h = (trep7h[:, :].rearrange("p (to k) -> p to k", k=FC)
                   .unsqueeze(1).broadcast_to([128, NP, FC, FC]))
            v.tensor_tensor(g3v, f2v, t7h, op=ADD)
            v.engine_nop()
            lo0 = (trep7t[:, :].rearrange("p (f k) -> p f k", k=FC)
                   .unsqueeze(1).broadcast_to([128, NP, FC, FC]))
            ov0 = ovn(NP)
            for to in range(FC):
                hi = (g3v[:, :, to, :].unsqueeze(2)
                      .broadcast_to([128, NP, FC, FC]))
                v.tensor_tensor(ov0[:, to], hi, lo0, op=ADD)
            maxred(Bp[:, :], NP)
            # chunk-0 pair 0 := tropical identity
            v.tensor_scalar_add(
                Bp[0:BL, :].rearrange("p (to j f) -> p to j f",
                                      j=NP, f=FC)[:, :, 0],
                identB[:, :].rearrange("p (to f) -> p to f", f=FC), 0.0)
            v.engine_nop()
            v.engine_nop()

            def copy_t(dstT, srcN, n):
                """dstT[j,x,y] = srcN-product M_j[y,x] (to-major src)."""
                o = dstT[:, :].rearrange("p (j x y) -> p j x y", x=FC, y=FC)
                i = srcN[:, :].rearrange("p (y j x) -> p j x y", j=n, x=FC)
                v.tensor_scalar_add(o, i, 0.0)
                v.engine_nop()
                v.engine_nop()

            copy_t(BpT, Bp, NP)

            # levels 1..4: all-fp16, dual-orientation sources
            for (dstN, dstT, srcN, srcT, n) in [
                    (T2, T2T, Bp, BpT, 8), (T3, T3T, T2, T2T, 4),
                    (T4, T4T, T3, T3T, 2), (Ee, None, T4, T4T, 1)]:
                m = 2 * n
                sv = srcN[:, :].rearrange("p (to j k) -> p j to k",
                                          j=m, k=FC)
                lo = (srcT[:, :].rearrange("p (j f k) -> p j f k",
                                           f=FC, k=FC)[:, 0:m:2])
                ovl = ovn(n)
                for to in range(FC):
                    hi = (sv[:, 1:m:2, to, :].unsqueeze(2)
                          .broadcast_to([128, n, FC, FC]))
                    v.tensor_tensor(ovl[:, to], hi, lo, op=ADD)
                maxred(dstN[:, :], n)
                if dstT is not None:
                    copy_t(dstT, dstN, n)
            v.engine_nop().then_inc(ev_sem, 1)

            # A matrices (f32) for the phase-3 fills; overlaps the gather
            fv = (feat_sp[:, :].rearrange("p (to t) -> p t to", to=LC)
                  [:, :, 0:FC].unsqueeze(3).broadcast_to([128, S, FC, FC]))
            tv = (trep7[:, :].rearrange("p (to k) -> p to k", k=FC)
                  .unsqueeze(1).broadcast_to([128, S, FC, FC]))
            v.tensor_tensor(Av, tv, fv, op=ADD)
            v.engine_nop()

            # phase 2: carries D_{c+1} = E_c (x) D_c  (b-partition layout)
            v.wait_ge(g_sem, 32)
            egv = Eg[:, :].rearrange("p (c to k) -> p c to k", to=FC, k=FC)
            s2 = sc2[:, :].rearrange("p (to k) -> p to k", k=FC)
            for c in range(C - 1):
                if c == 8:
                    v.wait_ge(gh_sem, 32)
                din = (Dg[:, c * FC:(c + 1) * FC]
                       .rearrange("p (a k) -> p a k", a=1)
                       .broadcast_to([BL, FC, FC]))
                v.tensor_tensor(s2, egv[:, c], din, op=ADD)
                v.tensor_reduce(Dg[:, (c + 1) * FC:(c + 2) * FC], s2,
                                axis=AXX, op=MAX)
                v.engine_nop()
                if c == 7:
                    v.engine_nop().then_inc(p2h_sem, 1)  # D_0..D_8 final
                else:
                    v.engine_nop()
            v.engine_nop().then_inc(p2_sem, 1)

            # phase 3: re-scan. 4 serial oct steps, then three bulk fills.
            v.wait_ge(sct_sem, 48)
            s3 = scw[:, 0:FC * FC].rearrange("p (to k) -> p to k", k=FC)
            t3q = T3[:, :].rearrange("p (to i f) -> p i to f", i=4, f=FC)
            for i in range(4):
                din = (delta[:, 8 * i * FC:(8 * i + 1) * FC]
                       .rearrange("p (a k) -> p a k", a=1)
                       .broadcast_to([128, FC, FC]))
                v.tensor_tensor(s3, t3q[:, i], din, op=ADD)
                v.tensor_reduce(delta[:, (8 * i + 8) * FC:(8 * i + 9) * FC],
                                s3, axis=AXX, op=MAX)
                v.engine_nop()
                v.engine_nop()
            # quad-fill: local_{8i+3} = T2_{2i} (x) local_{8i-1}, i=0..3
            t2e = (T2[:, :].rearrange("p (to i k) -> p i to k",
                                      i=8, k=FC)[:, 0:8:2])
            d8q = (dlt[:, 0:S + 1:8, :][:, 0:4].unsqueeze(2)
                   .broadcast_to([128, 4, FC, FC]))
            o4q = (scw[:, 0:4 * FC * FC]
                   .rearrange("p (i to k) -> p i to k", to=FC, k=FC))
            v.tensor_tensor(o4q, t2e, d8q, op=ADD)
            v.tensor_reduce(dlt[:, 4:S:8, :], o4q, axis=AXX, op=MAX)
            v.engine_nop()
            v.engine_nop()
            # pairs-fill: local_{4i+1} = B_{2i} (x) local_{4i-1}, i=0..7
            bq = (Bp[:, :].rearrange("p (to j k) -> p j to k",
                                     j=NP, k=FC)[:, 0:NP:2])
            dq = (dlt[:, 0:S:4, :].unsqueeze(2)
                  .broadcast_to([128, 8, FC, FC]))
            oq = (scw[:, 0:8 * FC * FC]
                  .rearrange("p (i to k) -> p i to k", to=FC, k=FC))
            v.tensor_tensor(oq, bq, dq, op=ADD)
            v.tensor_reduce(dlt[:, 2:S:4, :], oq, axis=AXX, op=MAX)
            v.engine_nop()
            v.engine_nop()
            # evens: local_{2j} = A_{2j} (x) local_{2j-1} for all j at once
            ae = (Av[:, 0:S:2, :, :])                          # [p,16,7,7]
            de = (dlt[:, 0:S:2, :].unsqueeze(2)
                  .broadcast_to([128, NP, FC, FC]))
            oe = (scw[:, 0:NP * FC * FC]
                  .rearrange("p (j to k) -> p j to k", to=FC, k=FC))
            v.tensor_tensor(oe, ae, de, op=ADD)
            v.tensor_reduce(dlt[:, 1:S:2, :], oe, axis=AXX, op=MAX)
            v.engine_nop().then_inc(dv_sem, 1)

    return nc


_PROG = None


def _get_prog():
    global _PROG
    if _PROG is None:
        _PROG = build_program()
    return _PROG


def make_in_maps(hidden_states, W, b, transitions):
    import ml_dtypes
    hs = np.asarray(hidden_states, np.float32)
    W = np.asarray(W, np.float32)
    bb = np.asarray(b, np.float32)
    trans = np.asarray(transitions, np.float32)

    Wc = W[:, LAB]                                       # [768, 8]
    wk = np.ascontiguousarray(Wc.reshape(KC, 128, LC).transpose(1, 0, 2)
                              ).reshape(128, KC * LC).astype(ml_dtypes.bfloat16)
    t7 = (trans + bb[:, None])[0:FC, 0:FC]               # [7, 7]
    trep7 = np.ascontiguousarray(
        np.broadcast_to(t7.reshape(1, FC * FC), (128, FC * FC))).astype(
            np.float32)
    trep7h = trep7.astype(np.float16)
    trep7t = np.ascontiguousarray(
        np.broadcast_to(t7.T.reshape(1, FC * FC),
                        (128, FC * FC))).astype(np.float16)
    d7c = np.ascontiguousarray(
        np.broadcast_to((trans[0:FC, START] + bb[0:FC])[None, :],
                        (BL, FC))).astype(np.float32)
    idm = np.where(np.eye(FC, dtype=bool), 0.0, NEG)
    identB = np.ascontiguousarray(
        np.broadcast_to(idm.reshape(1, FC * FC),
                        (BL, FC * FC))).astype(np.float16)

    in_maps = []
    for c in range(NC):
        shard = hs[c * BL:(c + 1) * BL]                 # [8, 512, 768]
        hsT = np.ascontiguousarray(shard.transpose(0, 2, 1)).astype(
            ml_dtypes.bfloat16)                         # [8, 768, 512]
        in_maps.append({"hsT": hsT, "wk": wk, "trep7": trep7,
                        "trep7h": trep7h, "trep7t": trep7t,
                        "d7c": d7c, "identB": identB})
    return in_maps


def decode(ddel_list, f511_list, transitions, bias):
    """ddel [128, 224] f32 per core, f511 [8(to), 8(b)] -> path [64,512]."""
    trans = np.asarray(transitions, np.float32)
    bias = np.asarray(bias, np.float32)
    lab = np.array(LAB, np.int64)
    t8 = trans[LAB][:, 0:FC] + bias[LAB][:, None]        # [8to, 7k]
    delta = np.empty((B, T, FC), np.float32)
    d8 = np.empty((B, LC), np.float32)
    for c in range(NC):
        dd = ddel_list[c].reshape(C, BL, S, FC)          # [(c,b), j, f]
        delta[c * BL:(c + 1) * BL] = (dd.transpose(1, 0, 2, 3)
                                      .reshape(BL, T, FC))
        d510 = delta[c * BL:(c + 1) * BL, T - 2, :]      # [b, 7]
        d8[c * BL:(c + 1) * BL] = ((t8[None, :, :] + d510[:, None, :])
                                   .max(-1) + f511_list[c].T)
    path = np.empty((B, T), np.int32)
    cur = lab[np.argmax(d8, axis=1)]                     # labels, may be 8
    path[:, T - 1] = cur
    for t in range(T - 1, 1, -1):
        cur = np.argmax(trans[cur, 0:FC] + delta[:, t - 1, :], axis=1)
        path[:, t - 1] = cur
    path[:, 0] = START
    return path


def kernel(hidden_states, W, b, transitions):
    in_maps = make_in_maps(hidden_states, W, b, transitions)
    nc = _get_prog()
    res = run_bass_kernel_spmd(nc, in_maps, list(range(NC))).results
    return decode([res[c]["ddel"] for c in range(NC)],
                  [res[c]["f511"] for c in range(NC)], transitions, b)
